# revision 1
# baseline (speedup 1.0000x reference)
"""nn_KimiDecoderLayer on 8 TRN2 NeuronCores, fully on-device.

Sharding: TP4 over heads x DP2 over batch. Core c in 0..3: batch 0, heads
(2c, 2c+1); core c+4: batch 1, same heads. Projections, short conv, gated
delta-rule recurrence (chunked, exact hierarchical decay factorization
matching the reference), gated RMS norm run head-local; o_proj runs
row-parallel after an in-kernel AllToAll of the normed outputs.
"""
import numpy as np
import ml_dtypes

B, T, H = 2, 1024, 2048
NH, D = 8, 128
P = NH * D
K = 4
EPS = 1e-6
SCALE = D ** -0.5
C = 128
JH = 10
NCORES = 8
ROWS = 1024          # rows per core (one batch)
NCH = 8              # chunks per head

LAST_EXEC_NS = None
_CACHE = {}

BF16 = ml_dtypes.bfloat16


def _split_excess_waits(nc, mybir, max_waits=1):
    """walrus in this env rejects >1 sem-wait per instruction; hoist extras
    onto preceding same-engine NoOps (semantically identical)."""
    n = 0
    for f in nc.m.functions:
        for blk in f.blocks:
            new = []
            changed = False
            for inst in blk.instructions:
                si = inst.sync_info
                if si is not None and si.on_wait is not None and len(si.on_wait) > max_waits:
                    waits = list(si.on_wait)
                    groups = [waits[i:i + max_waits] for i in range(0, len(waits), max_waits)]
                    for gi, g in enumerate(groups[:-1]):
                        new.append(mybir.InstNoOp(
                            name=f"{inst.name}-ws{gi}", engine=inst.engine,
                            sync_info=mybir.SyncInfo(on_wait=g, on_update=[]),
                            bass_nofuse=True))
                    inst.sync_info = mybir.SyncInfo(on_wait=groups[-1],
                                                    on_update=list(si.on_update))
                    changed = True
                    n += 1
                new.append(inst)
            if changed:
                blk.instructions = new
    return n


def _masks():
    def rect(bs):
        m = np.zeros((C, C), np.float32)
        for b0 in range(0, C, bs):
            m[b0 + bs // 2:b0 + bs, b0:b0 + bs // 2] = 1
        return m
    m32, m16 = rect(32), rect(16)
    mask8 = np.zeros((C, C), np.float32)
    for b0 in range(0, C, 8):
        mask8[b0:b0 + 8, b0:b0 + 8] = 1
    m8 = np.tril(mask8, -1)
    return m32, m16, m8


def build_nc(with_collective=True, debug_ogt=False, split_waits=True):
    import concourse.bass as bass
    import concourse.tile as tile
    from concourse import mybir

    f32 = mybir.dt.float32
    bf16 = mybir.dt.bfloat16

    nc = bass.Bass(num_devices=NCORES)
    d_xt = nc.dram_tensor("xt", (H, ROWS), bf16, kind="ExternalInput")
    d_wp = nc.dram_tensor("wp", (H, 1026), bf16, kind="ExternalInput")
    d_wfb = nc.dram_tensor("wfb", (128, 256), bf16, kind="ExternalInput")
    d_wgb = nc.dram_tensor("wgb", (128, 256), bf16, kind="ExternalInput")
    d_wo = nc.dram_tensor("wo", (P, H), bf16, kind="ExternalInput")
    d_cst = nc.dram_tensor("cst", (128, 32), f32, kind="ExternalInput")
    d_out = nc.dram_tensor("out", (256, H), f32, kind="ExternalOutput")
    d_ogt_dbg = None
    if debug_ogt:
        d_ogt_dbg = nc.dram_tensor("ogt_dbg", (256, ROWS), f32, kind="ExternalOutput")
    if with_collective:
        d_cin = nc.dram_tensor("a2a_in", (2 * P, 128), bf16, kind="Internal")
        d_cout = nc.dram_tensor("a2a_out", (2 * P, 128), bf16, kind="Internal")

    m32_np, m16_np, m8_np = _masks()
    d_m = {}
    d_mnp = {"m32": m32_np, "m16": m16_np, "m8": m8_np,
             "m32T": np.ascontiguousarray(m32_np.T),
             "m16T": np.ascontiguousarray(m16_np.T),
             "m8T": np.ascontiguousarray(m8_np.T)}
    for nm, arr in d_mnp.items():
        d_m[nm] = nc.inline_tensor(arr.astype(np.float32), name=nm)
    d_m4 = {}
    for nm in ("m32T", "m16T", "m8T"):
        arr4 = np.tile(d_mnp[nm], (1, 4))
        d_m4[nm] = nc.inline_tensor(arr4.astype(np.uint8), name=nm + "x4")
    d_id_bf = nc.inline_tensor(np.eye(128, dtype=BF16), name="idbf")
    d_id_f32 = nc.inline_tensor(np.eye(128, dtype=np.float32), name="idf32")
    d_ones_f32 = nc.inline_tensor(np.ones((128, 128), np.float32), name="ones32")
    d_ones_bf = nc.inline_tensor(np.ones((128, 1), BF16), name="onesbf")

    AF = mybir.ActivationFunctionType
    OP = mybir.AluOpType

    with tile.TileContext(nc) as tc:
        with tc.tile_pool(name="wpool", bufs=1) as wp:
            masksb = {}
            u8 = mybir.dt.uint8
            for nm in ("m32T", "m16T", "m8T"):
                mt = wp.tile([128, 512], u8, name=nm + "x4")
                nc.sync.dma_start(out=mt, in_=d_m4[nm][:, :])
                masksb[nm + "x4"] = mt
            idbf = wp.tile([128, 128], bf16, name="idbf")
            nc.sync.dma_start(out=idbf, in_=d_id_bf[:, :])
            idf32 = wp.tile([128, 128], f32, name="idf32")
            nc.sync.dma_start(out=idf32, in_=d_id_f32[:, :])
            ones32 = wp.tile([128, 128], f32, name="ones32")
            nc.sync.dma_start(out=ones32, in_=d_ones_f32[:, :])
            onesbf = wp.tile([128, 1], bf16, name="onesbf")
            nc.sync.dma_start(out=onesbf, in_=d_ones_bf[:, :])
            cst = wp.tile([128, 32], f32, name="cst")
            nc.sync.dma_start(out=cst, in_=d_cst[:, :])

            qc = [wp.tile([128, ROWS], bf16, name=f"qc{i}") for i in range(2)]
            kc = [wp.tile([128, ROWS], bf16, name=f"kc{i}") for i in range(2)]
            vc = [wp.tile([128, ROWS], bf16, name=f"vc{i}") for i in range(2)]
            gT = [wp.tile([128, ROWS], f32, name=f"gT{i}") for i in range(2)]
            sig = [wp.tile([128, ROWS], bf16, name=f"sig{i}") for i in range(2)]
            betaT = wp.tile([2, ROWS], f32, name="betaT")
            betaN = wp.tile([128, 16], f32, name="betaN")
            ogT = [wp.tile([128, ROWS], bf16, name=f"ogT{i}") for i in range(2)]

            # ================= phase 1: projections =================
            with tc.tile_pool(name="ph1", bufs=1) as p1, \
                 tc.tile_pool(name="ph1t", bufs=2) as p1t, \
                 tc.tile_pool(name="ps1", bufs=2, space="PSUM") as ps1:
                xt = p1.tile([128, 16 * ROWS], bf16, name="xt")
                for kt in range(16):
                    nc.sync.dma_start(out=xt[:, kt * ROWS:(kt + 1) * ROWS],
                                      in_=d_xt[kt * 128:(kt + 1) * 128, :])
                wpj = p1.tile([128, 16 * 1026], bf16, name="wpj")
                for kt in range(16):
                    nc.sync.dma_start(out=wpj[:, kt * 1026:(kt + 1) * 1026],
                                      in_=d_wp[kt * 128:(kt + 1) * 128, :])
                wfb = p1.tile([128, 256], bf16, name="wfb")
                nc.sync.dma_start(out=wfb, in_=d_wfb[:, :])
                wgb = p1.tile([128, 256], bf16, name="wgb")
                nc.sync.dma_start(out=wgb, in_=d_wgb[:, :])

                qr = [p1.tile([128, ROWS + 3], bf16, name=f"qr{i}") for i in range(2)]
                kr = [p1.tile([128, ROWS + 3], bf16, name=f"kr{i}") for i in range(2)]
                vr = [p1.tile([128, ROWS + 3], bf16, name=f"vr{i}") for i in range(2)]
                fafT = p1.tile([128, ROWS], bf16, name="fafT")
                fagT = p1.tile([128, ROWS], bf16, name="fagT")
                raws = [qr[0], qr[1], kr[0], kr[1], vr[0], vr[1]]
                for t_ in raws:
                    nc.vector.memset(t_[:, 0:3], 0.0)

                for mc in (6, 7, 8, 0, 2, 4, 1, 3, 5):
                    m0 = mc * 128
                    msz = 2 if mc == 8 else 128
                    for nn in range(ROWS // 512):
                        ps = ps1.tile([128, 512], f32, name="pj")
                        for kt in range(16):
                            nc.tensor.matmul(
                                ps[0:msz, :],
                                lhsT=wpj[:, kt * 1026 + m0: kt * 1026 + m0 + msz],
                                rhs=xt[:, kt * ROWS + nn * 512: kt * ROWS + (nn + 1) * 512],
                                start=(kt == 0), stop=(kt == 15))
                        if mc < 6:
                            nc.scalar.copy(out=raws[mc][:, 3 + nn * 512: 3 + (nn + 1) * 512],
                                           in_=ps[:, :])
                        elif mc == 6:
                            nc.scalar.copy(out=fafT[:, nn * 512:(nn + 1) * 512], in_=ps[:, :])
                        elif mc == 7:
                            nc.scalar.copy(out=fagT[:, nn * 512:(nn + 1) * 512], in_=ps[:, :])
                        else:
                            nc.scalar.activation(out=betaT[:, nn * 512:(nn + 1) * 512],
                                                 in_=ps[0:2, :], func=AF.Sigmoid)

                for c in range(NCH):
                    psb = ps1.tile([128, 512], f32, name="pj")
                    nc.tensor.matmul(psb[0:128, 0:2],
                                     lhsT=betaT[0:2, c * 128:(c + 1) * 128],
                                     rhs=idf32[0:2, 0:2], is_transpose=True)
                    nc.scalar.copy(out=betaN[:, 2 * c:2 * c + 2], in_=psb[:, 0:2])

                for i in range(2):
                    for raw, out_t, cbase in ((qr[i], qc[i], 12 * i),
                                              (kr[i], kc[i], 12 * i + 4),
                                              (vr[i], vc[i], 12 * i + 8)):
                        acc = p1t.tile([128, ROWS], f32, name="cacc")
                        nc.scalar.mul(out=acc, in_=raw[:, 0:ROWS],
                                      mul=cst[:, cbase:cbase + 1])
                        for j in range(1, K):
                            acc2 = p1t.tile([128, ROWS], f32, name="cacc")
                            nc.vector.scalar_tensor_tensor(
                                out=acc2, in0=raw[:, j:j + ROWS],
                                scalar=cst[:, cbase + j:cbase + j + 1],
                                in1=acc, op0=OP.mult, op1=OP.add)
                            acc = acc2
                        sg = p1t.tile([128, ROWS], bf16, name="csig")
                        nc.scalar.activation(out=sg, in_=acc, func=AF.Sigmoid)
                        nc.vector.tensor_tensor(out=out_t, in0=acc, in1=sg, op=OP.mult)

                for i in range(2):
                    for nn in range(ROWS // 512):
                        ps = ps1.tile([128, 512], f32, name="pj")
                        nc.tensor.matmul(ps[:, :], lhsT=wfb[:, 128 * i:128 * (i + 1)],
                                         rhs=fafT[:, nn * 512:(nn + 1) * 512],
                                         start=True, stop=True)
                        tmp2 = p1t.tile([128, 512], f32, name="gtmp2")
                        nc.scalar.activation(out=tmp2, in_=ps[:, :], func=AF.Exp,
                                             bias=cst[:, 24 + i:25 + i])
                        tmp3 = p1t.tile([128, 512], f32, name="gtmp3")
                        nc.scalar.activation(out=tmp3, in_=tmp2, func=AF.Ln, bias=1.0)
                        nc.vector.tensor_scalar(out=gT[i][:, nn * 512:(nn + 1) * 512],
                                                in0=tmp3, scalar1=cst[:, 26 + i:27 + i],
                                                scalar2=None, op0=OP.mult)

                for i in range(2):
                    for hf in range(2):
                        ps = ps1.tile([128, 512], f32, name="pj")
                        for cc in range(4):
                            c = hf * 4 + cc
                            nc.tensor.matmul(ps[:, cc * 128:(cc + 1) * 128],
                                             lhsT=fagT[:, c * 128:(c + 1) * 128],
                                             rhs=wgb[:, 128 * i:128 * (i + 1)],
                                             start=True, stop=True)
                        nc.scalar.activation(out=sig[i][:, hf * 512:(hf + 1) * 512],
                                             in_=ps[:, :], func=AF.Sigmoid)

            # ================= phase 2: recurrence =================
            pwo_cm = tc.tile_pool(name="phwo", bufs=1)
            pwo = pwo_cm.__enter__()
            wo = None
            if with_collective:
                wo = pwo.tile([128, 8 * H], bf16, name="wo")
                for kt in range(8):
                    nc.sync.dma_start(out=wo[:, kt * H:(kt + 1) * H],
                                      in_=d_wo[kt * 128:(kt + 1) * 128, :])
            with tc.tile_pool(name="ph2u", bufs=1) as p2u, \
                 tc.tile_pool(name="ph2", bufs=1) as p2, \
                 tc.tile_pool(name="ph2t", bufs=2) as p2t, \
                 tc.tile_pool(name="ph2s", bufs=3) as p2s, \
                 tc.tile_pool(name="ph2w", bufs=2) as p2w, \
                 tc.tile_pool(name="psGm", bufs=1, space="PSUM") as psGmP, \
                 tc.tile_pool(name="psGx", bufs=2, space="PSUM") as psGxP, \
                 tc.tile_pool(name="psX", bufs=2, space="PSUM") as psXP, \
                 tc.tile_pool(name="psB", bufs=3, space="PSUM") as psBP:
                PH = [{} for _ in range(2)]
                for i in range(2):
                    ph = PH[i]
                    GTz = p2.tile([128, NCH * 129], f32, name="GTz")
                    nc.vector.memset(
                        GTz[:, :].rearrange("p (c z) -> p c z", z=129)[:, :, 0:1], 0.0)
                    for c in range(NCH):
                        nc.vector.tensor_tensor_scan(
                            out=GTz[:, c * 129 + 1:(c + 1) * 129],
                            data0=ones32[:, 0:128],
                            data1=gT[i][:, c * 128:(c + 1) * 128],
                            initial=0.0, op0=OP.mult, op1=OP.add)
                    GT3 = GTz[:, :].rearrange("p (c z) -> p c z", z=129)
                    gview = GT3[:, :, 1:129]
                    zview = GT3[:, :, 0:128]

                    eGT = p2.tile([128, ROWS], bf16, name="eGT")
                    nc.scalar.activation(
                        out=eGT[:, :].rearrange("p (c t) -> p c t", t=128),
                        in_=gview, func=AF.Exp)
                    eGlast = p2.tile([128, NCH], f32, name=f"eGl{i}")
                    nc.scalar.activation(
                        out=eGlast[:, :].rearrange("p (c o) -> p c o", o=1),
                        in_=GT3[:, :, 128:129], func=AF.Exp)
                    ph["eGlast"] = eGlast

                    kn = p2.tile([128, ROWS], bf16, name="kn")
                    sq = p2u.tile([128, ROWS], f32, name="sq")
                    nc.scalar.activation(out=sq, in_=kc[i], func=AF.Square)
                    rrow = p2t.tile([1, ROWS], f32, name="rrow")
                    for hf in range(2):
                        psq = psGxP.tile([128, 512], f32, name="psGx")
                        nc.tensor.matmul(psq[0:1, :], lhsT=ones32[:, 0:1],
                                         rhs=sq[:, hf * 512:(hf + 1) * 512],
                                         start=True, stop=True)
                        nc.scalar.activation(out=rrow[0:1, hf * 512:(hf + 1) * 512],
                                             in_=psq[0:1, :], func=AF.Sqrt,
                                             bias=cst[0:1, 28:29])
                    nc.vector.reciprocal(out=rrow, in_=rrow)
                    for hf in range(2):
                        psbc = psGxP.tile([128, 512], f32, name="psGx")
                        for cc in range(4):
                            c = hf * 4 + cc
                            nc.tensor.matmul(psbc[:, cc * 128:(cc + 1) * 128],
                                             lhsT=ones32[0:1, 0:128],
                                             rhs=rrow[0:1, c * 128:(c + 1) * 128],
                                             start=True, stop=True)
                        nc.vector.tensor_tensor(out=kn[:, hf * 512:(hf + 1) * 512],
                                                in0=kc[i][:, hf * 512:(hf + 1) * 512],
                                                in1=psbc, op=OP.mult)
                    sqq = p2u.tile([128, ROWS], f32, name="sq")
                    nc.scalar.activation(out=sqq, in_=qc[i], func=AF.Square)
                    psrq = psBP.tile([128, 128], f32, name="psB")
                    for c in range(NCH):
                        nc.tensor.matmul(psrq[:, c:c + 1],
                                         lhsT=sqq[:, c * 128:(c + 1) * 128],
                                         rhs=ones32[:, 0:1],
                                         start=True, stop=True)
                    rqn = p2.tile([128, NCH], f32, name=f"rqn{i}")
                    nc.scalar.activation(out=rqn, in_=psrq[:, 0:NCH], func=AF.Sqrt,
                                         scale=float(D), bias=cst[:, 29:30])
                    nc.vector.reciprocal(out=rqn, in_=rqn)
                    ph["rqn"] = rqn
                    qs = qc[i]

                    KeGT = p2.tile([128, ROWS], bf16, name=f"KeGT{i}")
                    nc.vector.tensor_tensor(out=KeGT, in0=kn, in1=eGT, op=OP.mult)
                    QeGsT = p2.tile([128, ROWS], bf16, name=f"QeG{i}")
                    nc.vector.tensor_tensor(out=QeGsT, in0=qs, in1=eGT, op=OP.mult)
                    qk = p2.tile([128, ROWS], bf16, name="qk")
                    nc.vector.tensor_tensor(out=qk, in0=qs, in1=kn, op=OP.mult)
                    ph["KeGT"] = KeGT
                    ph["QeGsT"] = QeGsT

                    etT = p2t.tile([128, ROWS], bf16, name="etT")
                    for c in range(NCH):
                        nc.scalar.activation(out=etT[:, c * 128:(c + 1) * 128],
                                             in_=GTz[:, c * 129 + 1:(c + 1) * 129],
                                             func=AF.Exp, scale=-1.0,
                                             bias=GTz[:, c * 129 + 128:c * 129 + 129])
                    KetT = p2t.tile([128, ROWS], bf16, name="KetT")
                    nc.vector.tensor_tensor(out=KetT, in0=kn, in1=etT, op=OP.mult)
                    KetN = p2.tile([128, ROWS], bf16, name=f"KetN{i}")
                    VN = p2.tile([128, ROWS], bf16, name=f"VN{i}")
                    ph["KetN"] = KetN
                    ph["VN"] = VN
                    for c in range(NCH):
                        nc.sync.dma_start_transpose(
                            out=KetN[:, c * 128:(c + 1) * 128],
                            in_=KetT[:, c * 128:(c + 1) * 128])
                        nc.sync.dma_start_transpose(
                            out=VN[:, c * 128:(c + 1) * 128],
                            in_=vc[i][:, c * 128:(c + 1) * 128])

                    lvl_ops = []
                    for li, bs in enumerate((128, 64, 32, 16, 8)):
                        nb = 128 // bs
                        cen_off = bs // 2 if bs > 8 else 0
                        g4 = gview.rearrange("p c (nb bs) -> p c nb bs", bs=bs)
                        z4 = zview.rearrange("p c (nb bs) -> p c nb bs", bs=bs)
                        cen = z4[:, :, :, cen_off:cen_off + 1].broadcast_to(
                            [128, NCH, nb, bs])
                        dlt = p2u.tile([128, ROWS], f32, name="dlt")
                        nc.vector.tensor_tensor(
                            out=dlt[:, :].rearrange("p (c nb bs) -> p c nb bs",
                                                    c=NCH, bs=bs),
                            in0=g4, in1=cen, op=OP.subtract)
                        el = p2u.tile([128, ROWS], bf16, name="el")
                        er = p2u.tile([128, ROWS], bf16, name="er")
                        KEL = p2.tile([128, ROWS], bf16, name=f"KEL{li}")
                        KER = p2.tile([128, ROWS], bf16, name=f"KER{li}")
                        QEL = p2.tile([128, ROWS], bf16, name=f"QEL{li}")

                        def hv(t_, lohi):
                            v = t_[:, :].rearrange("p (c nb b2) -> p c nb b2",
                                                   c=NCH, b2=bs)
                            return (v[:, :, :, bs // 2:bs] if lohi else
                                    v[:, :, :, 0:bs // 2])

                        if li <= 1:
                            nc.scalar.activation(out=hv(el, 1), in_=hv(dlt, 1),
                                                 func=AF.Exp)
                            nc.scalar.activation(out=hv(er, 0), in_=hv(dlt, 0),
                                                 func=AF.Exp, scale=-1.0)
                            nc.vector.tensor_tensor(out=hv(KEL, 1), in0=hv(kn, 1),
                                                    in1=hv(el, 1), op=OP.mult)
                            nc.vector.tensor_tensor(out=hv(KER, 0), in0=hv(kn, 0),
                                                    in1=hv(er, 0), op=OP.mult)
                            nc.vector.tensor_tensor(out=hv(QEL, 1), in0=hv(qs, 1),
                                                    in1=hv(el, 1), op=OP.mult)
                        elif bs > 8:
                            nc.vector.memset(hv(KEL, 0), 0.0)
                            nc.vector.memset(hv(QEL, 0), 0.0)
                            nc.scalar.activation(out=hv(el, 1), in_=hv(dlt, 1),
                                                 func=AF.Exp)
                            nc.scalar.activation(out=er, in_=dlt, func=AF.Exp,
                                                 scale=-1.0)
                            nc.vector.tensor_tensor(out=hv(KEL, 1), in0=hv(kn, 1),
                                                    in1=hv(el, 1), op=OP.mult)
                            nc.vector.tensor_tensor(out=KER, in0=kn, in1=er,
                                                    op=OP.mult)
                            nc.vector.tensor_tensor(out=hv(QEL, 1), in0=hv(qs, 1),
                                                    in1=hv(el, 1), op=OP.mult)
                        else:
                            nc.scalar.activation(out=el, in_=dlt, func=AF.Exp)
                            tmx = p2u.tile([128, ROWS], f32, name="tmx")
                            nc.vector.tensor_scalar(out=tmx, in0=dlt, scalar1=-85.0,
                                                    scalar2=None, op0=OP.max)
                            nc.scalar.activation(out=er, in_=tmx, func=AF.Exp,
                                                 scale=-1.0)
                            nc.vector.tensor_tensor(out=KEL, in0=kn, in1=el, op=OP.mult)
                            nc.vector.tensor_tensor(out=KER, in0=kn, in1=er, op=OP.mult)
                            nc.vector.tensor_tensor(out=QEL, in0=qs, in1=el, op=OP.mult)
                        lvl_ops.append((KEL, KER, QEL))

                    TmT = [p2.tile([128, 128], bf16, name=f"TmT{i}_{c}")
                           for c in range(NCH)]
                    AqT = [p2.tile([128, 128], bf16, name=f"AqT{i}_{c}")
                           for c in range(NCH)]
                    ph["TmT"] = TmT
                    ph["AqT"] = AqT
                    Nb = p2.tile([128, ROWS], bf16, name="Nb")
                    KEL0, KER0, QEL0 = lvl_ops[0]
                    KEL1, KER1, QEL1 = lvl_ops[1]
                    KEL2, KER2, QEL2 = lvl_ops[2]
                    KEL3, KER3, QEL3 = lvl_ops[3]
                    KEL8, KER8, QEL8 = lvl_ops[4]
                    for half in range(2):
                        for orient in range(2):
                            psMain = psGmP.tile([128, 512], f32, name="psGm")
                            nc.vector.memset(psMain, 0.0)
                            psLs = []
                            for lvl_idx, (KA, KB) in enumerate((
                                    (KER2, KEL2) if orient == 0 else (KER2, QEL2),
                                    (KER3, KEL3) if orient == 0 else (KER3, QEL3),
                                    (KER8, KEL8) if orient == 0 else (KER8, QEL8))):
                                psL = psGxP.tile([128, 512], f32, name="psGx")
                                for cc in range(4):
                                    c = half * 4 + cc
                                    sl = slice(c * 128, (c + 1) * 128)
                                    nc.tensor.matmul(
                                        psL[:, cc * 128:(cc + 1) * 128],
                                        lhsT=KA[:, sl], rhs=KB[:, sl],
                                        start=True, stop=True)
                                psLs.append(psL)
                            RL0 = KEL0 if orient == 0 else QEL0
                            RL1 = KEL1 if orient == 0 else QEL1
                            for cc in range(4):
                                c = half * 4 + cc
                                o0 = cc * 128
                                nc.tensor.matmul(
                                    psMain[0:64, o0 + 64:o0 + 128],
                                    lhsT=KER0[:, c * 128:c * 128 + 64],
                                    rhs=RL0[:, c * 128 + 64:c * 128 + 128],
                                    start=True, stop=True)
                                nc.tensor.matmul(
                                    psMain[0:32, o0 + 32:o0 + 64],
                                    lhsT=KER1[:, c * 128:c * 128 + 32],
                                    rhs=RL1[:, c * 128 + 32:c * 128 + 64],
                                    start=True, stop=True)
                                nc.tensor.matmul(
                                    psMain[64:96, o0 + 96:o0 + 128],
                                    lhsT=KER1[:, c * 128 + 64:c * 128 + 96],
                                    rhs=RL1[:, c * 128 + 96:c * 128 + 128],
                                    start=True, stop=True)
                            a0 = p2w.tile([128, 512], f32, name="a0")
                            nc.scalar.copy(out=a0, in_=psMain)
                            a1 = p2w.tile([128, 512], f32, name="a1")
                            nc.vector.select(out=a1, mask=masksb["m32Tx4"],
                                             on_true=psLs[0], on_false=a0)
                            a3 = p2w.tile([128, 512], f32, name="a1")
                            nc.vector.select(out=a3, mask=masksb["m16Tx4"],
                                             on_true=psLs[1], on_false=a1)
                            gsum = p2w.tile([128, 512], f32, name="gsum")
                            nc.vector.select(out=gsum, mask=masksb["m8Tx4"],
                                             on_true=psLs[2], on_false=a3)
                            for cc in range(4):
                                c = half * 4 + cc
                                sl = slice(c * 128, (c + 1) * 128)
                                gsl = slice(cc * 128, (cc + 1) * 128)
                                if orient == 0:
                                    nc.scalar.mul(
                                        out=Nb[:, c * 128:(c + 1) * 128],
                                        in_=gsum[:, gsl],
                                        mul=betaN[:, 2 * c + i:2 * c + i + 1])
                                else:
                                    psd = psBP.tile([128, 128], f32, name="psB")
                                    nc.tensor.matmul(psd[:, 0:1], lhsT=qk[:, sl],
                                                     rhs=onesbf[:, 0:1],
                                                     start=True, stop=True)
                                    tmpA = p2s.tile([128, 128], bf16, name="tmpA")
                                    nc.scalar.copy(out=tmpA, in_=gsum[:, gsl])
                                    nc.vector.scalar_tensor_tensor(
                                        out=AqT[c], in0=idbf, scalar=psd[:, 0:1],
                                        in1=tmpA, op0=OP.mult, op1=OP.add)

                    idx3 = idbf[:, :].unsqueeze(1).broadcast_to([128, 4, 128])
                    for hf in range(2):
                        xprev = [idbf] * 4
                        for j in range(JH):
                            psXb = psXP.tile([128, 512], f32, name="psX")
                            for cc in range(4):
                                c = hf * 4 + cc
                                nc.tensor.matmul(psXb[:, cc * 128:(cc + 1) * 128],
                                                 lhsT=Nb[:, c * 128:(c + 1) * 128],
                                                 rhs=xprev[cc], start=True, stop=True)
                            xnb = p2s.tile([128, 512], bf16, name="xh")
                            nc.vector.scalar_tensor_tensor(
                                out=xnb[:, :].rearrange("p (c t) -> p c t", t=128),
                                in0=idx3, scalar=1.0,
                                in1=psXb[:, :].rearrange("p (c t) -> p c t", t=128),
                                op0=OP.mult, op1=OP.subtract)
                            xprev = [xnb[:, cc * 128:(cc + 1) * 128] for cc in range(4)]
                        for cc in range(4):
                            c = hf * 4 + cc
                            nc.sync.dma_start_transpose(out=TmT[c][:, :], in_=xprev[cc])

                    ogg = p2t.tile([128, ROWS], f32, name=f"ogg{i}")
                    ph["ogg"] = ogg
                    Sf = p2.tile([128, 128], f32, name=f"Sf{i}")
                    Sb = p2.tile([128, 128], bf16, name=f"Sb{i}")
                    nc.vector.memset(Sf, 0.0)
                    nc.vector.memset(Sb, 0.0)
                    ph["Sf"] = Sf
                    ph["Sb"] = Sb
                # serial chain for this head
                    for c in range(NCH):
                        sl = slice(c * 128, (c + 1) * 128)
                        Sf, Sb = ph["Sf"], ph["Sb"]
                        psKS = psBP.tile([128, 128], f32, name="psB")
                        nc.tensor.matmul(psKS[:, :], lhsT=ph["KeGT"][:, sl], rhs=Sb,
                                         start=True, stop=True)
                        Rr = p2s.tile([128, 128], bf16, name="Rr")
                        nc.vector.tensor_tensor(out=Rr, in0=ph["VN"][:, sl], in1=psKS,
                                                op=OP.subtract)
                        psY = psBP.tile([128, 128], f32, name="psB")
                        nc.tensor.matmul(psY[:, :], lhsT=ph["TmT"][c], rhs=Rr,
                                         start=True, stop=True)
                        Dl = p2s.tile([128, 128], bf16, name="Dl")
                        nc.scalar.mul(out=Dl, in_=psY,
                                      mul=betaN[:, 2 * c + i:2 * c + i + 1])
                        psO = psBP.tile([128, 128], f32, name="psB")
                        nc.tensor.matmul(psO[:, :], lhsT=ph["QeGsT"][:, sl], rhs=Sb,
                                         start=True, stop=False)
                        nc.tensor.matmul(psO[:, :], lhsT=ph["AqT"][c], rhs=Dl,
                                         start=False, stop=True)
                        psS = psBP.tile([128, 128], f32, name="psB")
                        nc.tensor.matmul(psS[:, :], lhsT=ph["KetN"][:, sl], rhs=Dl,
                                         start=True, stop=True)
                        nc.vector.scalar_tensor_tensor(
                            out=Sf, in0=Sf, scalar=ph["eGlast"][:, c:c + 1],
                            in1=psS, op0=OP.mult, op1=OP.add)
                        nc.scalar.copy(out=Sb, in_=Sf)
                        nc.vector.scalar_tensor_tensor(
                            out=ph["ogg"][:, sl], in0=psO,
                            scalar=ph["rqn"][:, c:c + 1],
                            in1=sig[i][:, sl], op0=OP.mult, op1=OP.mult)

                # batched gated-RMS + transpose per head
                    ogg = ph["ogg"]
                    sqo = p2u.tile([128, ROWS], f32, name="sqo")
                    nc.scalar.activation(out=sqo, in_=ogg, func=AF.Square)
                    ssr = p2s.tile([128, NCH], f32, name="ssr")
                    nc.vector.tensor_reduce(
                        out=ssr, in_=sqo[:, :].rearrange("p (c t) -> p c t", t=128),
                        axis=mybir.AxisListType.X, op=OP.add)
                    rmsr = p2s.tile([128, NCH], f32, name="rmsr")
                    nc.scalar.activation(out=rmsr, in_=ssr, func=AF.Sqrt,
                                         scale=1.0 / D, bias=cst[:, 28:29])
                    nc.vector.reciprocal(out=rmsr, in_=rmsr)
                    ogn = p2u.tile([128, ROWS], bf16, name="ogn")
                    nc.vector.tensor_tensor(
                        out=ogn[:, :].rearrange("p (c t) -> p c t", t=128),
                        in0=ogg[:, :].rearrange("p (c t) -> p c t", t=128),
                        in1=rmsr[:, :].unsqueeze(2).broadcast_to([128, NCH, 128]),
                        op=OP.mult)
                    for c in range(NCH):
                        sl = slice(c * 128, (c + 1) * 128)
                        nc.sync.dma_start_transpose(out=ogT[i][:, sl], in_=ogn[:, sl])


            # ================= phase 3: A2A + o_proj =================
            if debug_ogt:
                with tc.tile_pool(name="dbg", bufs=1) as dbp:
                    for i in range(2):
                        tmp = dbp.tile([128, ROWS], f32, name="dbgcp")
                        nc.scalar.copy(out=tmp, in_=ogT[i])
                        nc.sync.dma_start(out=d_ogt_dbg[i * 128:(i + 1) * 128, :], in_=tmp)

            if with_collective:
                with tc.tile_pool(name="ph3", bufs=1) as p3, \
                     tc.tile_pool(name="ph3t", bufs=2) as p3t, \
                     tc.tile_pool(name="ps3", bufs=2, space="PSUM") as ps3:
                    # shard j (rows 256j..256j+256) = [h0 | h1] cols 128j..128j+128
                    for j in range(8):
                        for i in range(2):
                            nc.sync.dma_start(
                                out=d_cin[256 * j + 128 * i:256 * j + 128 * (i + 1), :],
                                in_=ogT[i][:, 128 * j:128 * (j + 1)])
                    nc.gpsimd.collective_compute(
                        "AllToAll", mybir.AluOpType.bypass,
                        ins=[d_cin[:, :]], outs=[d_cout[:, :]],
                        replica_groups=[list(range(8))])
                    # cout rows [0:1024) = batch-0 og (heads 0..7), [1024:2048) batch-1
                    oga = p3.tile([128, 16 * 128], bf16, name="oga")
                    for kt in range(16):
                        nc.sync.dma_start(out=oga[:, kt * 128:(kt + 1) * 128],
                                          in_=d_cout[kt * 128:(kt + 1) * 128, :])
                    for b2 in range(2):
                        for nn2 in range(H // 512):
                            pso = ps3.tile([128, 512], f32, name="pso")
                            for kt in range(8):
                                nc.tensor.matmul(
                                    pso[:, :],
                                    lhsT=oga[:, (b2 * 8 + kt) * 128:
                                             (b2 * 8 + kt + 1) * 128],
                                    rhs=wo[:, kt * H + nn2 * 512:
                                           kt * H + (nn2 + 1) * 512],
                                    start=(kt == 0), stop=(kt == 7))
                            osb = p3t.tile([128, 512], f32, name="osb")
                            nc.scalar.copy(out=osb, in_=pso)
                            nc.sync.dma_start(
                                out=d_out[b2 * 128:(b2 + 1) * 128,
                                          nn2 * 512:(nn2 + 1) * 512],
                                in_=osb)

            pwo_cm.__exit__(None, None, None)

    if split_waits:
        _split_excess_waits(nc, mybir)
    return nc


def _host_inputs(inputs):
    x = np.asarray(inputs['hidden_states'], np.float32)
    Wq, Wk, Wv = (np.asarray(inputs[k], np.float32) for k in ('Wq', 'Wk', 'Wv'))
    conv_q, conv_k, conv_v = (np.asarray(inputs[k], np.float32)
                              for k in ('conv_q', 'conv_k', 'conv_v'))
    A_log = np.asarray(inputs['A_log'], np.float32)
    dt_bias = np.asarray(inputs['dt_bias'], np.float32)
    Wfa, Wfb = np.asarray(inputs['Wfa'], np.float32), np.asarray(inputs['Wfb'], np.float32)
    Wb = np.asarray(inputs['Wb'], np.float32)
    Wga, Wgb = np.asarray(inputs['Wga'], np.float32), np.asarray(inputs['Wgb'], np.float32)
    nw = np.asarray(inputs['norm_weight'], np.float32)
    Wo = np.asarray(inputs['Wo'], np.float32)
    WoT_folded = np.ascontiguousarray((Wo * np.tile(nw, NH)[None, :]).T)

    in_maps = []
    for cidx in range(NCORES):
        b, hp = cidx // 4, cidx % 4
        heads = (2 * hp, 2 * hp + 1)
        xT = np.ascontiguousarray(x[b].T)
        Wsl = []
        for h in heads: Wsl.append(Wq[128 * h:128 * (h + 1)])
        for h in heads: Wsl.append(Wk[128 * h:128 * (h + 1)])
        for h in heads: Wsl.append(Wv[128 * h:128 * (h + 1)])
        Wsl.append(Wfa); Wsl.append(Wga)
        Wsl.append(Wb[list(heads)])
        WprojT = np.ascontiguousarray(np.concatenate(Wsl, axis=0).T)
        consts = np.zeros((128, 32), np.float32)
        for i, h in enumerate(heads):
            consts[:, 12 * i + 0:12 * i + 4] = conv_q[128 * h:128 * (h + 1)]
            consts[:, 12 * i + 4:12 * i + 8] = conv_k[128 * h:128 * (h + 1)]
            consts[:, 12 * i + 8:12 * i + 12] = conv_v[128 * h:128 * (h + 1)]
            consts[:, 24 + i] = dt_bias[128 * h:128 * (h + 1)]
            consts[:, 26 + i] = -np.exp(A_log[h])
        consts[:, 28] = EPS
        consts[:, 29] = D * EPS
        wfb2 = np.concatenate([Wfb[128 * h:128 * (h + 1)].T for h in heads], axis=1)
        wgb2 = np.concatenate([Wgb[128 * h:128 * (h + 1)].T for h in heads], axis=1)
        in_maps.append({
            "xt": xT.astype(BF16),
            "wp": WprojT.astype(BF16),
            "wfb": np.ascontiguousarray(wfb2).astype(BF16),
            "wgb": np.ascontiguousarray(wgb2).astype(BF16),
            "wo": WoT_folded.astype(BF16),
            "cst": consts,
        })
    return in_maps


def _estimate_exec_ns(nc):
    """Best-effort single-core HW time estimate from the instruction cost
    model (NTFF profiling is unavailable under the axon client)."""
    try:
        from concourse.timeline_sim import TimelineSim
        return int(TimelineSim(nc, trace=False).simulate())
    except Exception:
        return None


def kernel(**inputs):
    global LAST_EXEC_NS
    from concourse.bass_utils import run_bass_kernel_spmd

    if "nc" not in _CACHE:
        _CACHE["nc"] = build_nc()
    nc = _CACHE["nc"]
    in_maps = _host_inputs(inputs)
    res = run_bass_kernel_spmd(nc, in_maps, core_ids=list(range(NCORES)), trace=False)
    if res.exec_time_ns is not None:
        LAST_EXEC_NS = res.exec_time_ns
    else:
        LAST_EXEC_NS = _estimate_exec_ns(nc)
    out = np.zeros((B * T, H), np.float32)
    for cidx in range(NCORES):
        r = res.results[cidx]["out"]
        out[128 * cidx:128 * (cidx + 1)] = r[0:128]
        out[T + 128 * cidx:T + 128 * (cidx + 1)] = r[128:256]
    return out



# revision 2
# speedup vs baseline: 1.0332x; 1.0332x over previous
"""nn_KimiDecoderLayer on 8 TRN2 NeuronCores, fully on-device.

Sharding: TP4 over heads x DP2 over batch. Core c in 0..3: batch 0, heads
(2c, 2c+1); core c+4: batch 1, same heads. Projections, short conv, gated
delta-rule recurrence (chunked, exact hierarchical decay factorization
matching the reference), gated RMS norm run head-local; o_proj runs
row-parallel after an in-kernel AllToAll of the normed outputs.
"""
import numpy as np
import ml_dtypes

B, T, H = 2, 1024, 2048
NH, D = 8, 128
P = NH * D
K = 4
EPS = 1e-6
SCALE = D ** -0.5
C = 128
JH = 8
NCORES = 8
ROWS = 1024          # rows per core (one batch)
NCH = 8              # chunks per head

LAST_EXEC_NS = None
_CACHE = {}

BF16 = ml_dtypes.bfloat16


def _split_excess_waits(nc, mybir, max_waits=1):
    """walrus in this env rejects >1 sem-wait per instruction; hoist extras
    onto preceding same-engine NoOps (semantically identical)."""
    n = 0
    for f in nc.m.functions:
        for blk in f.blocks:
            new = []
            changed = False
            for inst in blk.instructions:
                si = inst.sync_info
                if si is not None and si.on_wait is not None and len(si.on_wait) > max_waits:
                    waits = list(si.on_wait)
                    groups = [waits[i:i + max_waits] for i in range(0, len(waits), max_waits)]
                    for gi, g in enumerate(groups[:-1]):
                        new.append(mybir.InstNoOp(
                            name=f"{inst.name}-ws{gi}", engine=inst.engine,
                            sync_info=mybir.SyncInfo(on_wait=g, on_update=[]),
                            bass_nofuse=True))
                    inst.sync_info = mybir.SyncInfo(on_wait=groups[-1],
                                                    on_update=list(si.on_update))
                    changed = True
                    n += 1
                new.append(inst)
            if changed:
                blk.instructions = new
    return n


def _masks():
    def rect(bs):
        m = np.zeros((C, C), np.float32)
        for b0 in range(0, C, bs):
            m[b0 + bs // 2:b0 + bs, b0:b0 + bs // 2] = 1
        return m
    m32, m16 = rect(32), rect(16)
    mask8 = np.zeros((C, C), np.float32)
    for b0 in range(0, C, 8):
        mask8[b0:b0 + 8, b0:b0 + 8] = 1
    m8 = np.tril(mask8, -1)
    return m32, m16, m8


def build_nc(with_collective=True, debug_ogt=False, split_waits=True):
    import concourse.bass as bass
    import concourse.tile as tile
    from concourse import mybir

    f32 = mybir.dt.float32
    bf16 = mybir.dt.bfloat16

    nc = bass.Bass(num_devices=NCORES)
    d_xt = nc.dram_tensor("xt", (H, ROWS), bf16, kind="ExternalInput")
    d_wp = nc.dram_tensor("wp", (H, 1026), bf16, kind="ExternalInput")
    d_wfb = nc.dram_tensor("wfb", (128, 256), bf16, kind="ExternalInput")
    d_wgb = nc.dram_tensor("wgb", (128, 256), bf16, kind="ExternalInput")
    d_wo = nc.dram_tensor("wo", (P, H), bf16, kind="ExternalInput")
    d_cst = nc.dram_tensor("cst", (128, 32), f32, kind="ExternalInput")
    d_out = nc.dram_tensor("out", (256, H), f32, kind="ExternalOutput")
    d_ogt_dbg = None
    if debug_ogt:
        d_ogt_dbg = nc.dram_tensor("ogt_dbg", (256, ROWS), f32, kind="ExternalOutput")
    if with_collective:
        d_cin = nc.dram_tensor("a2a_in", (2 * P, 128), bf16, kind="Internal")
        d_cout = nc.dram_tensor("a2a_out", (2 * P, 128), bf16, kind="Internal")

    m32_np, m16_np, m8_np = _masks()
    d_m = {}
    d_mnp = {"m32": m32_np, "m16": m16_np, "m8": m8_np,
             "m32T": np.ascontiguousarray(m32_np.T),
             "m16T": np.ascontiguousarray(m16_np.T),
             "m8T": np.ascontiguousarray(m8_np.T)}
    for nm, arr in d_mnp.items():
        d_m[nm] = nc.inline_tensor(arr.astype(np.float32), name=nm)
    d_m4 = {}
    for nm in ("m32T", "m16T", "m8T"):
        arr4 = np.tile(d_mnp[nm], (1, 4))
        d_m4[nm] = nc.inline_tensor(arr4.astype(np.uint8), name=nm + "x4")
    d_id_bf = nc.inline_tensor(np.eye(128, dtype=BF16), name="idbf")
    d_id_f32 = nc.inline_tensor(np.eye(128, dtype=np.float32), name="idf32")
    d_ones_f32 = nc.inline_tensor(np.ones((128, 128), np.float32), name="ones32")
    d_ones_bf = nc.inline_tensor(np.ones((128, 1), BF16), name="onesbf")

    AF = mybir.ActivationFunctionType
    OP = mybir.AluOpType

    with tile.TileContext(nc) as tc:
        with tc.tile_pool(name="wpool", bufs=1) as wp:
            masksb = {}
            u8 = mybir.dt.uint8
            for nm in ("m32T", "m16T", "m8T"):
                mt = wp.tile([128, 512], u8, name=nm + "x4")
                nc.sync.dma_start(out=mt, in_=d_m4[nm][:, :])
                masksb[nm + "x4"] = mt
            idbf = wp.tile([128, 128], bf16, name="idbf")
            nc.sync.dma_start(out=idbf, in_=d_id_bf[:, :])
            idf32 = wp.tile([128, 128], f32, name="idf32")
            nc.sync.dma_start(out=idf32, in_=d_id_f32[:, :])
            ones32 = wp.tile([128, 128], f32, name="ones32")
            nc.sync.dma_start(out=ones32, in_=d_ones_f32[:, :])
            onesbf = wp.tile([128, 1], bf16, name="onesbf")
            nc.sync.dma_start(out=onesbf, in_=d_ones_bf[:, :])
            cst = wp.tile([128, 32], f32, name="cst")
            nc.sync.dma_start(out=cst, in_=d_cst[:, :])

            qc = [wp.tile([128, ROWS], bf16, name=f"qc{i}") for i in range(2)]
            kc = [wp.tile([128, ROWS], bf16, name=f"kc{i}") for i in range(2)]
            vc = [wp.tile([128, ROWS], bf16, name=f"vc{i}") for i in range(2)]
            gT = [wp.tile([128, ROWS], f32, name=f"gT{i}") for i in range(2)]
            sig = [wp.tile([128, ROWS], bf16, name=f"sig{i}") for i in range(2)]
            betaT = wp.tile([2, ROWS], f32, name="betaT")
            betaN = wp.tile([128, 16], f32, name="betaN")
            ogT = [wp.tile([128, ROWS], bf16, name=f"ogT{i}") for i in range(2)]

            # ================= phase 1: projections =================
            with tc.tile_pool(name="ph1", bufs=1) as p1, \
                 tc.tile_pool(name="ph1t", bufs=2) as p1t, \
                 tc.tile_pool(name="ps1", bufs=2, space="PSUM") as ps1:
                xt = p1.tile([128, 16 * ROWS], bf16, name="xt")
                for kt in range(16):
                    nc.sync.dma_start(out=xt[:, kt * ROWS:(kt + 1) * ROWS],
                                      in_=d_xt[kt * 128:(kt + 1) * 128, :])
                wpj = p1.tile([128, 16 * 1026], bf16, name="wpj")
                for kt in range(16):
                    nc.sync.dma_start(out=wpj[:, kt * 1026:(kt + 1) * 1026],
                                      in_=d_wp[kt * 128:(kt + 1) * 128, :])
                wfb = p1.tile([128, 256], bf16, name="wfb")
                nc.sync.dma_start(out=wfb, in_=d_wfb[:, :])
                wgb = p1.tile([128, 256], bf16, name="wgb")
                nc.sync.dma_start(out=wgb, in_=d_wgb[:, :])

                qr = [p1.tile([128, ROWS + 3], bf16, name=f"qr{i}") for i in range(2)]
                kr = [p1.tile([128, ROWS + 3], bf16, name=f"kr{i}") for i in range(2)]
                vr = [p1.tile([128, ROWS + 3], bf16, name=f"vr{i}") for i in range(2)]
                fafT = p1.tile([128, ROWS], bf16, name="fafT")
                fagT = p1.tile([128, ROWS], bf16, name="fagT")
                raws = [qr[0], qr[1], kr[0], kr[1], vr[0], vr[1]]
                for t_ in raws:
                    nc.vector.memset(t_[:, 0:3], 0.0)

                for mc in (6, 7, 8, 0, 2, 4, 1, 3, 5):
                    m0 = mc * 128
                    msz = 2 if mc == 8 else 128
                    for nn in range(ROWS // 512):
                        ps = ps1.tile([128, 512], f32, name="pj")
                        for kt in range(16):
                            nc.tensor.matmul(
                                ps[0:msz, :],
                                lhsT=wpj[:, kt * 1026 + m0: kt * 1026 + m0 + msz],
                                rhs=xt[:, kt * ROWS + nn * 512: kt * ROWS + (nn + 1) * 512],
                                start=(kt == 0), stop=(kt == 15))
                        if mc < 6:
                            nc.scalar.copy(out=raws[mc][:, 3 + nn * 512: 3 + (nn + 1) * 512],
                                           in_=ps[:, :])
                        elif mc == 6:
                            nc.scalar.copy(out=fafT[:, nn * 512:(nn + 1) * 512], in_=ps[:, :])
                        elif mc == 7:
                            nc.scalar.copy(out=fagT[:, nn * 512:(nn + 1) * 512], in_=ps[:, :])
                        else:
                            nc.scalar.activation(out=betaT[:, nn * 512:(nn + 1) * 512],
                                                 in_=ps[0:2, :], func=AF.Sigmoid)

                for c in range(NCH):
                    psb = ps1.tile([128, 512], f32, name="pj")
                    nc.tensor.matmul(psb[0:128, 0:2],
                                     lhsT=betaT[0:2, c * 128:(c + 1) * 128],
                                     rhs=idf32[0:2, 0:2], is_transpose=True)
                    nc.scalar.copy(out=betaN[:, 2 * c:2 * c + 2], in_=psb[:, 0:2])

                for i in range(2):
                    for raw, out_t, cbase in ((qr[i], qc[i], 12 * i),
                                              (kr[i], kc[i], 12 * i + 4),
                                              (vr[i], vc[i], 12 * i + 8)):
                        acc = p1t.tile([128, ROWS], f32, name="cacc")
                        nc.scalar.mul(out=acc, in_=raw[:, 0:ROWS],
                                      mul=cst[:, cbase:cbase + 1])
                        for j in range(1, K):
                            acc2 = p1t.tile([128, ROWS], f32, name="cacc")
                            nc.vector.scalar_tensor_tensor(
                                out=acc2, in0=raw[:, j:j + ROWS],
                                scalar=cst[:, cbase + j:cbase + j + 1],
                                in1=acc, op0=OP.mult, op1=OP.add)
                            acc = acc2
                        sg = p1t.tile([128, ROWS], bf16, name="csig")
                        nc.scalar.activation(out=sg, in_=acc, func=AF.Sigmoid)
                        nc.vector.tensor_tensor(out=out_t, in0=acc, in1=sg, op=OP.mult)

                for i in range(2):
                    for nn in range(ROWS // 512):
                        ps = ps1.tile([128, 512], f32, name="pj")
                        nc.tensor.matmul(ps[:, :], lhsT=wfb[:, 128 * i:128 * (i + 1)],
                                         rhs=fafT[:, nn * 512:(nn + 1) * 512],
                                         start=True, stop=True)
                        tmp2 = p1t.tile([128, 512], f32, name="gtmp2")
                        nc.scalar.activation(out=tmp2, in_=ps[:, :], func=AF.Exp,
                                             bias=cst[:, 24 + i:25 + i])
                        tmp3 = p1t.tile([128, 512], f32, name="gtmp3")
                        nc.scalar.activation(out=tmp3, in_=tmp2, func=AF.Ln, bias=1.0)
                        nc.vector.tensor_scalar(out=gT[i][:, nn * 512:(nn + 1) * 512],
                                                in0=tmp3, scalar1=cst[:, 26 + i:27 + i],
                                                scalar2=None, op0=OP.mult)

                for i in range(2):
                    for hf in range(2):
                        ps = ps1.tile([128, 512], f32, name="pj")
                        for cc in range(4):
                            c = hf * 4 + cc
                            nc.tensor.matmul(ps[:, cc * 128:(cc + 1) * 128],
                                             lhsT=fagT[:, c * 128:(c + 1) * 128],
                                             rhs=wgb[:, 128 * i:128 * (i + 1)],
                                             start=True, stop=True)
                        nc.scalar.activation(out=sig[i][:, hf * 512:(hf + 1) * 512],
                                             in_=ps[:, :], func=AF.Sigmoid)

            # ================= phase 2: recurrence =================
            pwo_cm = tc.tile_pool(name="phwo", bufs=1)
            pwo = pwo_cm.__enter__()
            wo = None
            if with_collective:
                wo = pwo.tile([128, 8 * H], bf16, name="wo")
                for kt in range(8):
                    nc.sync.dma_start(out=wo[:, kt * H:(kt + 1) * H],
                                      in_=d_wo[kt * 128:(kt + 1) * 128, :])
            with tc.tile_pool(name="ph2u", bufs=1) as p2u, \
                 tc.tile_pool(name="ph2", bufs=1) as p2, \
                 tc.tile_pool(name="ph2t", bufs=2) as p2t, \
                 tc.tile_pool(name="ph2s", bufs=3) as p2s, \
                 tc.tile_pool(name="ph2w", bufs=2) as p2w, \
                 tc.tile_pool(name="psGm", bufs=1, space="PSUM") as psGmP, \
                 tc.tile_pool(name="psGx", bufs=2, space="PSUM") as psGxP, \
                 tc.tile_pool(name="psX", bufs=2, space="PSUM") as psXP, \
                 tc.tile_pool(name="psB", bufs=3, space="PSUM") as psBP:
                PH = [{} for _ in range(2)]
                for i in range(2):
                    ph = PH[i]
                    GTz = p2.tile([128, NCH * 129], f32, name="GTz")
                    nc.vector.memset(
                        GTz[:, :].rearrange("p (c z) -> p c z", z=129)[:, :, 0:1], 0.0)
                    for c in range(NCH):
                        nc.vector.tensor_tensor_scan(
                            out=GTz[:, c * 129 + 1:(c + 1) * 129],
                            data0=ones32[:, 0:128],
                            data1=gT[i][:, c * 128:(c + 1) * 128],
                            initial=0.0, op0=OP.mult, op1=OP.add)
                    GT3 = GTz[:, :].rearrange("p (c z) -> p c z", z=129)
                    gview = GT3[:, :, 1:129]
                    zview = GT3[:, :, 0:128]

                    eGT = p2.tile([128, ROWS], bf16, name="eGT")
                    nc.scalar.activation(
                        out=eGT[:, :].rearrange("p (c t) -> p c t", t=128),
                        in_=gview, func=AF.Exp)
                    eGlast = p2.tile([128, NCH], f32, name=f"eGl{i}")
                    nc.scalar.activation(
                        out=eGlast[:, :].rearrange("p (c o) -> p c o", o=1),
                        in_=GT3[:, :, 128:129], func=AF.Exp)
                    ph["eGlast"] = eGlast

                    kn = p2.tile([128, ROWS], bf16, name="kn")
                    sq = p2u.tile([128, ROWS], f32, name="sq")
                    nc.scalar.activation(out=sq, in_=kc[i], func=AF.Square)
                    rrow = p2t.tile([1, ROWS], f32, name="rrow")
                    for hf in range(2):
                        psq = psGxP.tile([128, 512], f32, name="psGx")
                        nc.tensor.matmul(psq[0:1, :], lhsT=ones32[:, 0:1],
                                         rhs=sq[:, hf * 512:(hf + 1) * 512],
                                         start=True, stop=True)
                        nc.scalar.activation(out=rrow[0:1, hf * 512:(hf + 1) * 512],
                                             in_=psq[0:1, :], func=AF.Sqrt,
                                             bias=cst[0:1, 28:29])
                    nc.vector.reciprocal(out=rrow, in_=rrow)
                    for hf in range(2):
                        psbc = psGxP.tile([128, 512], f32, name="psGx")
                        for cc in range(4):
                            c = hf * 4 + cc
                            nc.tensor.matmul(psbc[:, cc * 128:(cc + 1) * 128],
                                             lhsT=ones32[0:1, 0:128],
                                             rhs=rrow[0:1, c * 128:(c + 1) * 128],
                                             start=True, stop=True)
                        nc.vector.tensor_tensor(out=kn[:, hf * 512:(hf + 1) * 512],
                                                in0=kc[i][:, hf * 512:(hf + 1) * 512],
                                                in1=psbc, op=OP.mult)
                    sqq = p2u.tile([128, ROWS], f32, name="sq")
                    nc.scalar.activation(out=sqq, in_=qc[i], func=AF.Square)
                    psrq = psBP.tile([128, 128], f32, name="psB")
                    for c in range(NCH):
                        nc.tensor.matmul(psrq[:, c:c + 1],
                                         lhsT=sqq[:, c * 128:(c + 1) * 128],
                                         rhs=ones32[:, 0:1],
                                         start=True, stop=True)
                    rqn = p2.tile([128, NCH], f32, name=f"rqn{i}")
                    nc.scalar.activation(out=rqn, in_=psrq[:, 0:NCH], func=AF.Sqrt,
                                         scale=float(D), bias=cst[:, 29:30])
                    nc.vector.reciprocal(out=rqn, in_=rqn)
                    ph["rqn"] = rqn
                    qs = qc[i]

                    KeGT = p2.tile([128, ROWS], bf16, name=f"KeGT{i}")
                    nc.vector.tensor_tensor(out=KeGT, in0=kn, in1=eGT, op=OP.mult)
                    QeGsT = p2.tile([128, ROWS], bf16, name=f"QeG{i}")
                    nc.vector.tensor_tensor(out=QeGsT, in0=qs, in1=eGT, op=OP.mult)
                    qk = p2.tile([128, ROWS], bf16, name="qk")
                    nc.vector.tensor_tensor(out=qk, in0=qs, in1=kn, op=OP.mult)
                    ph["KeGT"] = KeGT
                    ph["QeGsT"] = QeGsT

                    etT = p2t.tile([128, ROWS], bf16, name="etT")
                    for c in range(NCH):
                        nc.scalar.activation(out=etT[:, c * 128:(c + 1) * 128],
                                             in_=GTz[:, c * 129 + 1:(c + 1) * 129],
                                             func=AF.Exp, scale=-1.0,
                                             bias=GTz[:, c * 129 + 128:c * 129 + 129])
                    KetT = p2t.tile([128, ROWS], bf16, name="KetT")
                    nc.vector.tensor_tensor(out=KetT, in0=kn, in1=etT, op=OP.mult)
                    KetN = p2.tile([128, ROWS], bf16, name=f"KetN{i}")
                    VN = p2.tile([128, ROWS], bf16, name=f"VN{i}")
                    ph["KetN"] = KetN
                    ph["VN"] = VN
                    for c in range(NCH):
                        nc.sync.dma_start_transpose(
                            out=KetN[:, c * 128:(c + 1) * 128],
                            in_=KetT[:, c * 128:(c + 1) * 128])
                        nc.sync.dma_start_transpose(
                            out=VN[:, c * 128:(c + 1) * 128],
                            in_=vc[i][:, c * 128:(c + 1) * 128])

                    lvl_ops = []
                    for li, bs in enumerate((128, 64, 32, 16, 8)):
                        nb = 128 // bs
                        cen_off = bs // 2 if bs > 8 else 0
                        g4 = gview.rearrange("p c (nb bs) -> p c nb bs", bs=bs)
                        z4 = zview.rearrange("p c (nb bs) -> p c nb bs", bs=bs)
                        cen = z4[:, :, :, cen_off:cen_off + 1].broadcast_to(
                            [128, NCH, nb, bs])
                        dlt = p2u.tile([128, ROWS], f32, name="dlt")
                        nc.vector.tensor_tensor(
                            out=dlt[:, :].rearrange("p (c nb bs) -> p c nb bs",
                                                    c=NCH, bs=bs),
                            in0=g4, in1=cen, op=OP.subtract)
                        el = p2u.tile([128, ROWS], bf16, name="el")
                        er = p2u.tile([128, ROWS], bf16, name="er")
                        KEL = p2.tile([128, ROWS], bf16, name=f"KEL{li}")
                        KER = p2.tile([128, ROWS], bf16, name=f"KER{li}")
                        QEL = p2.tile([128, ROWS], bf16, name=f"QEL{li}")

                        def hv(t_, lohi):
                            v = t_[:, :].rearrange("p (c nb b2) -> p c nb b2",
                                                   c=NCH, b2=bs)
                            return (v[:, :, :, bs // 2:bs] if lohi else
                                    v[:, :, :, 0:bs // 2])

                        if li <= 1:
                            nc.scalar.activation(out=hv(el, 1), in_=hv(dlt, 1),
                                                 func=AF.Exp)
                            nc.scalar.activation(out=hv(er, 0), in_=hv(dlt, 0),
                                                 func=AF.Exp, scale=-1.0)
                            nc.vector.tensor_tensor(out=hv(KEL, 1), in0=hv(kn, 1),
                                                    in1=hv(el, 1), op=OP.mult)
                            nc.vector.tensor_tensor(out=hv(KER, 0), in0=hv(kn, 0),
                                                    in1=hv(er, 0), op=OP.mult)
                            nc.vector.tensor_tensor(out=hv(QEL, 1), in0=hv(qs, 1),
                                                    in1=hv(el, 1), op=OP.mult)
                        elif bs > 8:
                            nc.vector.memset(hv(KEL, 0), 0.0)
                            nc.vector.memset(hv(QEL, 0), 0.0)
                            nc.scalar.activation(out=hv(el, 1), in_=hv(dlt, 1),
                                                 func=AF.Exp)
                            nc.scalar.activation(out=er, in_=dlt, func=AF.Exp,
                                                 scale=-1.0)
                            nc.vector.tensor_tensor(out=hv(KEL, 1), in0=hv(kn, 1),
                                                    in1=hv(el, 1), op=OP.mult)
                            nc.vector.tensor_tensor(out=KER, in0=kn, in1=er,
                                                    op=OP.mult)
                            nc.vector.tensor_tensor(out=hv(QEL, 1), in0=hv(qs, 1),
                                                    in1=hv(el, 1), op=OP.mult)
                        else:
                            nc.scalar.activation(out=el, in_=dlt, func=AF.Exp)
                            tmx = p2u.tile([128, ROWS], f32, name="tmx")
                            nc.vector.tensor_scalar(out=tmx, in0=dlt, scalar1=-85.0,
                                                    scalar2=None, op0=OP.max)
                            nc.scalar.activation(out=er, in_=tmx, func=AF.Exp,
                                                 scale=-1.0)
                            nc.vector.tensor_tensor(out=KEL, in0=kn, in1=el, op=OP.mult)
                            nc.vector.tensor_tensor(out=KER, in0=kn, in1=er, op=OP.mult)
                            nc.vector.tensor_tensor(out=QEL, in0=qs, in1=el, op=OP.mult)
                        lvl_ops.append((KEL, KER, QEL))

                    TmT = [p2.tile([128, 128], bf16, name=f"TmT{i}_{c}")
                           for c in range(NCH)]
                    AqT = [p2.tile([128, 128], bf16, name=f"AqT{i}_{c}")
                           for c in range(NCH)]
                    ph["TmT"] = TmT
                    ph["AqT"] = AqT
                    Nb = p2.tile([128, ROWS], bf16, name="Nb")
                    KEL0, KER0, QEL0 = lvl_ops[0]
                    KEL1, KER1, QEL1 = lvl_ops[1]
                    KEL2, KER2, QEL2 = lvl_ops[2]
                    KEL3, KER3, QEL3 = lvl_ops[3]
                    KEL8, KER8, QEL8 = lvl_ops[4]
                    for half in range(2):
                        for orient in range(2):
                            psMain = psGmP.tile([128, 512], f32, name="psGm")
                            nc.vector.memset(psMain, 0.0)
                            psLs = []
                            for lvl_idx, (KA, KB) in enumerate((
                                    (KER2, KEL2) if orient == 0 else (KER2, QEL2),
                                    (KER3, KEL3) if orient == 0 else (KER3, QEL3),
                                    (KER8, KEL8) if orient == 0 else (KER8, QEL8))):
                                psL = psGxP.tile([128, 512], f32, name="psGx")
                                for cc in range(4):
                                    c = half * 4 + cc
                                    sl = slice(c * 128, (c + 1) * 128)
                                    nc.tensor.matmul(
                                        psL[:, cc * 128:(cc + 1) * 128],
                                        lhsT=KA[:, sl], rhs=KB[:, sl],
                                        start=True, stop=True)
                                psLs.append(psL)
                            RL0 = KEL0 if orient == 0 else QEL0
                            RL1 = KEL1 if orient == 0 else QEL1
                            for cc in range(4):
                                c = half * 4 + cc
                                o0 = cc * 128
                                nc.tensor.matmul(
                                    psMain[0:64, o0 + 64:o0 + 128],
                                    lhsT=KER0[:, c * 128:c * 128 + 64],
                                    rhs=RL0[:, c * 128 + 64:c * 128 + 128],
                                    start=True, stop=True)
                                nc.tensor.matmul(
                                    psMain[0:32, o0 + 32:o0 + 64],
                                    lhsT=KER1[:, c * 128:c * 128 + 32],
                                    rhs=RL1[:, c * 128 + 32:c * 128 + 64],
                                    start=True, stop=True)
                                nc.tensor.matmul(
                                    psMain[64:96, o0 + 96:o0 + 128],
                                    lhsT=KER1[:, c * 128 + 64:c * 128 + 96],
                                    rhs=RL1[:, c * 128 + 96:c * 128 + 128],
                                    start=True, stop=True)
                            a0 = p2w.tile([128, 512], f32, name="a0")
                            nc.scalar.copy(out=a0, in_=psMain)
                            a1 = p2w.tile([128, 512], f32, name="a1")
                            nc.vector.select(out=a1, mask=masksb["m32Tx4"],
                                             on_true=psLs[0], on_false=a0)
                            a3 = p2w.tile([128, 512], f32, name="a1")
                            nc.vector.select(out=a3, mask=masksb["m16Tx4"],
                                             on_true=psLs[1], on_false=a1)
                            gsum = p2w.tile([128, 512], f32, name="gsum")
                            nc.vector.select(out=gsum, mask=masksb["m8Tx4"],
                                             on_true=psLs[2], on_false=a3)
                            for cc in range(4):
                                c = half * 4 + cc
                                sl = slice(c * 128, (c + 1) * 128)
                                gsl = slice(cc * 128, (cc + 1) * 128)
                                if orient == 0:
                                    nc.scalar.mul(
                                        out=Nb[:, c * 128:(c + 1) * 128],
                                        in_=gsum[:, gsl],
                                        mul=betaN[:, 2 * c + i:2 * c + i + 1])
                                else:
                                    psd = psBP.tile([128, 128], f32, name="psB")
                                    nc.tensor.matmul(psd[:, 0:1], lhsT=qk[:, sl],
                                                     rhs=onesbf[:, 0:1],
                                                     start=True, stop=True)
                                    tmpA = p2s.tile([128, 128], bf16, name="tmpA")
                                    nc.scalar.copy(out=tmpA, in_=gsum[:, gsl])
                                    nc.vector.scalar_tensor_tensor(
                                        out=AqT[c], in0=idbf, scalar=psd[:, 0:1],
                                        in1=tmpA, op0=OP.mult, op1=OP.add)

                    idx3 = idbf[:, :].unsqueeze(1).broadcast_to([128, 4, 128])
                    for hf in range(2):
                        xprev = [idbf] * 4
                        for j in range(JH):
                            psXb = psXP.tile([128, 512], f32, name="psX")
                            for cc in range(4):
                                c = hf * 4 + cc
                                nc.tensor.matmul(psXb[:, cc * 128:(cc + 1) * 128],
                                                 lhsT=Nb[:, c * 128:(c + 1) * 128],
                                                 rhs=xprev[cc], start=True, stop=True)
                            xnb = p2s.tile([128, 512], bf16, name="xh")
                            nc.vector.scalar_tensor_tensor(
                                out=xnb[:, :].rearrange("p (c t) -> p c t", t=128),
                                in0=idx3, scalar=1.0,
                                in1=psXb[:, :].rearrange("p (c t) -> p c t", t=128),
                                op0=OP.mult, op1=OP.subtract)
                            xprev = [xnb[:, cc * 128:(cc + 1) * 128] for cc in range(4)]
                        for cc in range(4):
                            c = hf * 4 + cc
                            nc.sync.dma_start_transpose(out=TmT[c][:, :], in_=xprev[cc])

                    ogg = p2t.tile([128, ROWS], f32, name=f"ogg{i}")
                    ph["ogg"] = ogg
                    Sf = p2.tile([128, 128], f32, name=f"Sf{i}")
                    Sb = p2.tile([128, 128], bf16, name=f"Sb{i}")
                    nc.vector.memset(Sf, 0.0)
                    nc.vector.memset(Sb, 0.0)
                    ph["Sf"] = Sf
                    ph["Sb"] = Sb
                # serial chain for this head
                    for c in range(NCH):
                        sl = slice(c * 128, (c + 1) * 128)
                        Sf, Sb = ph["Sf"], ph["Sb"]
                        psKS = psBP.tile([128, 128], f32, name="psB")
                        nc.tensor.matmul(psKS[:, :], lhsT=ph["KeGT"][:, sl], rhs=Sb,
                                         start=True, stop=True)
                        Rr = p2s.tile([128, 128], bf16, name="Rr")
                        nc.vector.tensor_tensor(out=Rr, in0=ph["VN"][:, sl], in1=psKS,
                                                op=OP.subtract)
                        psY = psBP.tile([128, 128], f32, name="psB")
                        nc.tensor.matmul(psY[:, :], lhsT=ph["TmT"][c], rhs=Rr,
                                         start=True, stop=True)
                        Dl = p2s.tile([128, 128], bf16, name="Dl")
                        nc.scalar.mul(out=Dl, in_=psY,
                                      mul=betaN[:, 2 * c + i:2 * c + i + 1])
                        psO = psBP.tile([128, 128], f32, name="psB")
                        nc.tensor.matmul(psO[:, :], lhsT=ph["QeGsT"][:, sl], rhs=Sb,
                                         start=True, stop=False)
                        nc.tensor.matmul(psO[:, :], lhsT=ph["AqT"][c], rhs=Dl,
                                         start=False, stop=True)
                        psS = psBP.tile([128, 128], f32, name="psB")
                        nc.tensor.matmul(psS[:, :], lhsT=ph["KetN"][:, sl], rhs=Dl,
                                         start=True, stop=True)
                        nc.vector.scalar_tensor_tensor(
                            out=Sf, in0=Sf, scalar=ph["eGlast"][:, c:c + 1],
                            in1=psS, op0=OP.mult, op1=OP.add)
                        nc.scalar.copy(out=Sb, in_=Sf)
                        nc.vector.scalar_tensor_tensor(
                            out=ph["ogg"][:, sl], in0=psO,
                            scalar=ph["rqn"][:, c:c + 1],
                            in1=sig[i][:, sl], op0=OP.mult, op1=OP.mult)

                # batched gated-RMS + transpose per head
                    ogg = ph["ogg"]
                    sqo = p2u.tile([128, ROWS], f32, name="sqo")
                    nc.scalar.activation(out=sqo, in_=ogg, func=AF.Square)
                    ssr = p2s.tile([128, NCH], f32, name="ssr")
                    nc.vector.tensor_reduce(
                        out=ssr, in_=sqo[:, :].rearrange("p (c t) -> p c t", t=128),
                        axis=mybir.AxisListType.X, op=OP.add)
                    rmsr = p2s.tile([128, NCH], f32, name="rmsr")
                    nc.scalar.activation(out=rmsr, in_=ssr, func=AF.Sqrt,
                                         scale=1.0 / D, bias=cst[:, 28:29])
                    nc.vector.reciprocal(out=rmsr, in_=rmsr)
                    ogn = p2u.tile([128, ROWS], bf16, name="ogn")
                    nc.vector.tensor_tensor(
                        out=ogn[:, :].rearrange("p (c t) -> p c t", t=128),
                        in0=ogg[:, :].rearrange("p (c t) -> p c t", t=128),
                        in1=rmsr[:, :].unsqueeze(2).broadcast_to([128, NCH, 128]),
                        op=OP.mult)
                    for c in range(NCH):
                        sl = slice(c * 128, (c + 1) * 128)
                        nc.sync.dma_start_transpose(out=ogT[i][:, sl], in_=ogn[:, sl])


            # ================= phase 3: A2A + o_proj =================
            if debug_ogt:
                with tc.tile_pool(name="dbg", bufs=1) as dbp:
                    for i in range(2):
                        tmp = dbp.tile([128, ROWS], f32, name="dbgcp")
                        nc.scalar.copy(out=tmp, in_=ogT[i])
                        nc.sync.dma_start(out=d_ogt_dbg[i * 128:(i + 1) * 128, :], in_=tmp)

            if with_collective:
                with tc.tile_pool(name="ph3", bufs=1) as p3, \
                     tc.tile_pool(name="ph3t", bufs=2) as p3t, \
                     tc.tile_pool(name="ps3", bufs=2, space="PSUM") as ps3:
                    # shard j (rows 256j..256j+256) = [h0 | h1] cols 128j..128j+128
                    for j in range(8):
                        for i in range(2):
                            nc.sync.dma_start(
                                out=d_cin[256 * j + 128 * i:256 * j + 128 * (i + 1), :],
                                in_=ogT[i][:, 128 * j:128 * (j + 1)])
                    nc.gpsimd.collective_compute(
                        "AllToAll", mybir.AluOpType.bypass,
                        ins=[d_cin[:, :]], outs=[d_cout[:, :]],
                        replica_groups=[list(range(8))])
                    # cout rows [0:1024) = batch-0 og (heads 0..7), [1024:2048) batch-1
                    oga = p3.tile([128, 16 * 128], bf16, name="oga")
                    for kt in range(16):
                        nc.sync.dma_start(out=oga[:, kt * 128:(kt + 1) * 128],
                                          in_=d_cout[kt * 128:(kt + 1) * 128, :])
                    for b2 in range(2):
                        for nn2 in range(H // 512):
                            pso = ps3.tile([128, 512], f32, name="pso")
                            for kt in range(8):
                                nc.tensor.matmul(
                                    pso[:, :],
                                    lhsT=oga[:, (b2 * 8 + kt) * 128:
                                             (b2 * 8 + kt + 1) * 128],
                                    rhs=wo[:, kt * H + nn2 * 512:
                                           kt * H + (nn2 + 1) * 512],
                                    start=(kt == 0), stop=(kt == 7))
                            osb = p3t.tile([128, 512], f32, name="osb")
                            nc.scalar.copy(out=osb, in_=pso)
                            nc.sync.dma_start(
                                out=d_out[b2 * 128:(b2 + 1) * 128,
                                          nn2 * 512:(nn2 + 1) * 512],
                                in_=osb)

            pwo_cm.__exit__(None, None, None)

    if split_waits:
        _split_excess_waits(nc, mybir)
    return nc


def _host_inputs(inputs):
    x = np.asarray(inputs['hidden_states'], np.float32)
    Wq, Wk, Wv = (np.asarray(inputs[k], np.float32) for k in ('Wq', 'Wk', 'Wv'))
    conv_q, conv_k, conv_v = (np.asarray(inputs[k], np.float32)
                              for k in ('conv_q', 'conv_k', 'conv_v'))
    A_log = np.asarray(inputs['A_log'], np.float32)
    dt_bias = np.asarray(inputs['dt_bias'], np.float32)
    Wfa, Wfb = np.asarray(inputs['Wfa'], np.float32), np.asarray(inputs['Wfb'], np.float32)
    Wb = np.asarray(inputs['Wb'], np.float32)
    Wga, Wgb = np.asarray(inputs['Wga'], np.float32), np.asarray(inputs['Wgb'], np.float32)
    nw = np.asarray(inputs['norm_weight'], np.float32)
    Wo = np.asarray(inputs['Wo'], np.float32)
    WoT_folded = np.ascontiguousarray((Wo * np.tile(nw, NH)[None, :]).T)

    in_maps = []
    for cidx in range(NCORES):
        b, hp = cidx // 4, cidx % 4
        heads = (2 * hp, 2 * hp + 1)
        xT = np.ascontiguousarray(x[b].T)
        Wsl = []
        for h in heads: Wsl.append(Wq[128 * h:128 * (h + 1)])
        for h in heads: Wsl.append(Wk[128 * h:128 * (h + 1)])
        for h in heads: Wsl.append(Wv[128 * h:128 * (h + 1)])
        Wsl.append(Wfa); Wsl.append(Wga)
        Wsl.append(Wb[list(heads)])
        WprojT = np.ascontiguousarray(np.concatenate(Wsl, axis=0).T)
        consts = np.zeros((128, 32), np.float32)
        for i, h in enumerate(heads):
            consts[:, 12 * i + 0:12 * i + 4] = conv_q[128 * h:128 * (h + 1)]
            consts[:, 12 * i + 4:12 * i + 8] = conv_k[128 * h:128 * (h + 1)]
            consts[:, 12 * i + 8:12 * i + 12] = conv_v[128 * h:128 * (h + 1)]
            consts[:, 24 + i] = dt_bias[128 * h:128 * (h + 1)]
            consts[:, 26 + i] = -np.exp(A_log[h])
        consts[:, 28] = EPS
        consts[:, 29] = D * EPS
        wfb2 = np.concatenate([Wfb[128 * h:128 * (h + 1)].T for h in heads], axis=1)
        wgb2 = np.concatenate([Wgb[128 * h:128 * (h + 1)].T for h in heads], axis=1)
        in_maps.append({
            "xt": xT.astype(BF16),
            "wp": WprojT.astype(BF16),
            "wfb": np.ascontiguousarray(wfb2).astype(BF16),
            "wgb": np.ascontiguousarray(wgb2).astype(BF16),
            "wo": WoT_folded.astype(BF16),
            "cst": consts,
        })
    return in_maps


def _estimate_exec_ns(nc):
    """Best-effort single-core HW time estimate from the instruction cost
    model (NTFF profiling is unavailable under the axon client)."""
    try:
        from concourse.timeline_sim import TimelineSim
        return int(TimelineSim(nc, trace=False).simulate())
    except Exception:
        return None


def kernel(**inputs):
    global LAST_EXEC_NS
    from concourse.bass_utils import run_bass_kernel_spmd

    if "nc" not in _CACHE:
        _CACHE["nc"] = build_nc()
    nc = _CACHE["nc"]
    in_maps = _host_inputs(inputs)
    res = run_bass_kernel_spmd(nc, in_maps, core_ids=list(range(NCORES)), trace=False)
    if res.exec_time_ns is not None:
        LAST_EXEC_NS = res.exec_time_ns
    else:
        LAST_EXEC_NS = _estimate_exec_ns(nc)
    out = np.zeros((B * T, H), np.float32)
    for cidx in range(NCORES):
        r = res.results[cidx]["out"]
        out[128 * cidx:128 * (cidx + 1)] = r[0:128]
        out[T + 128 * cidx:T + 128 * (cidx + 1)] = r[128:256]
    return out



# revision 3
# speedup vs baseline: 1.0486x; 1.0149x over previous
"""nn_KimiDecoderLayer on 8 TRN2 NeuronCores, fully on-device.

Sharding: TP4 over heads x DP2 over batch. Core c in 0..3: batch 0, heads
(2c, 2c+1); core c+4: batch 1, same heads. Projections, short conv, gated
delta-rule recurrence (chunked, exact hierarchical decay factorization
matching the reference), gated RMS norm run head-local; o_proj runs
row-parallel after an in-kernel AllToAll of the normed outputs.
"""
import numpy as np
import ml_dtypes

B, T, H = 2, 1024, 2048
NH, D = 8, 128
P = NH * D
K = 4
EPS = 1e-6
SCALE = D ** -0.5
C = 128
JH = 6
NCORES = 8
ROWS = 1024          # rows per core (one batch)
NCH = 8              # chunks per head

LAST_EXEC_NS = None
_CACHE = {}

BF16 = ml_dtypes.bfloat16


def _split_excess_waits(nc, mybir, max_waits=1):
    """walrus in this env rejects >1 sem-wait per instruction; hoist extras
    onto preceding same-engine NoOps (semantically identical)."""
    n = 0
    for f in nc.m.functions:
        for blk in f.blocks:
            new = []
            changed = False
            for inst in blk.instructions:
                si = inst.sync_info
                if si is not None and si.on_wait is not None and len(si.on_wait) > max_waits:
                    waits = list(si.on_wait)
                    groups = [waits[i:i + max_waits] for i in range(0, len(waits), max_waits)]
                    for gi, g in enumerate(groups[:-1]):
                        new.append(mybir.InstNoOp(
                            name=f"{inst.name}-ws{gi}", engine=inst.engine,
                            sync_info=mybir.SyncInfo(on_wait=g, on_update=[]),
                            bass_nofuse=True))
                    inst.sync_info = mybir.SyncInfo(on_wait=groups[-1],
                                                    on_update=list(si.on_update))
                    changed = True
                    n += 1
                new.append(inst)
            if changed:
                blk.instructions = new
    return n


def _masks():
    def rect(bs):
        m = np.zeros((C, C), np.float32)
        for b0 in range(0, C, bs):
            m[b0 + bs // 2:b0 + bs, b0:b0 + bs // 2] = 1
        return m
    m32, m16 = rect(32), rect(16)
    mask8 = np.zeros((C, C), np.float32)
    for b0 in range(0, C, 8):
        mask8[b0:b0 + 8, b0:b0 + 8] = 1
    m8 = np.tril(mask8, -1)
    return m32, m16, m8


def build_nc(with_collective=True, debug_ogt=False, split_waits=True):
    import concourse.bass as bass
    import concourse.tile as tile
    from concourse import mybir

    f32 = mybir.dt.float32
    bf16 = mybir.dt.bfloat16

    nc = bass.Bass(num_devices=NCORES)
    d_xt = nc.dram_tensor("xt", (H, ROWS), bf16, kind="ExternalInput")
    d_wp = nc.dram_tensor("wp", (H, 1026), bf16, kind="ExternalInput")
    d_wfb = nc.dram_tensor("wfb", (128, 256), bf16, kind="ExternalInput")
    d_wgb = nc.dram_tensor("wgb", (128, 256), bf16, kind="ExternalInput")
    d_wo = nc.dram_tensor("wo", (P, H), bf16, kind="ExternalInput")
    d_cst = nc.dram_tensor("cst", (128, 32), f32, kind="ExternalInput")
    d_out = nc.dram_tensor("out", (256, H), f32, kind="ExternalOutput")
    d_ogt_dbg = None
    if debug_ogt:
        d_ogt_dbg = nc.dram_tensor("ogt_dbg", (256, ROWS), f32, kind="ExternalOutput")
    if with_collective:
        d_cin = nc.dram_tensor("a2a_in", (2 * P, 128), bf16, kind="Internal")
        d_cout = nc.dram_tensor("a2a_out", (2 * P, 128), bf16, kind="Internal")

    m32_np, m16_np, m8_np = _masks()
    d_m = {}
    d_mnp = {"m32": m32_np, "m16": m16_np, "m8": m8_np,
             "m32T": np.ascontiguousarray(m32_np.T),
             "m16T": np.ascontiguousarray(m16_np.T),
             "m8T": np.ascontiguousarray(m8_np.T)}
    for nm, arr in d_mnp.items():
        d_m[nm] = nc.inline_tensor(arr.astype(np.float32), name=nm)
    d_m4 = {}
    for nm in ("m32T", "m16T", "m8T"):
        arr4 = np.tile(d_mnp[nm], (1, 4))
        d_m4[nm] = nc.inline_tensor(arr4.astype(np.uint8), name=nm + "x4")
    d_id_bf = nc.inline_tensor(np.eye(128, dtype=BF16), name="idbf")
    d_id_f32 = nc.inline_tensor(np.eye(128, dtype=np.float32), name="idf32")
    d_ones_f32 = nc.inline_tensor(np.ones((128, 128), np.float32), name="ones32")
    d_ones_bf = nc.inline_tensor(np.ones((128, 1), BF16), name="onesbf")

    AF = mybir.ActivationFunctionType
    OP = mybir.AluOpType

    with tile.TileContext(nc) as tc:
        with tc.tile_pool(name="wpool", bufs=1) as wp:
            masksb = {}
            u8 = mybir.dt.uint8
            for nm in ("m32T", "m16T", "m8T"):
                mt = wp.tile([128, 512], u8, name=nm + "x4")
                nc.sync.dma_start(out=mt, in_=d_m4[nm][:, :])
                masksb[nm + "x4"] = mt
            idbf = wp.tile([128, 128], bf16, name="idbf")
            nc.sync.dma_start(out=idbf, in_=d_id_bf[:, :])
            idf32 = wp.tile([128, 128], f32, name="idf32")
            nc.sync.dma_start(out=idf32, in_=d_id_f32[:, :])
            ones32 = wp.tile([128, 128], f32, name="ones32")
            nc.sync.dma_start(out=ones32, in_=d_ones_f32[:, :])
            onesbf = wp.tile([128, 1], bf16, name="onesbf")
            nc.sync.dma_start(out=onesbf, in_=d_ones_bf[:, :])
            cst = wp.tile([128, 32], f32, name="cst")
            nc.sync.dma_start(out=cst, in_=d_cst[:, :])

            qc = [wp.tile([128, ROWS], bf16, name=f"qc{i}") for i in range(2)]
            kc = [wp.tile([128, ROWS], bf16, name=f"kc{i}") for i in range(2)]
            vc = [wp.tile([128, ROWS], bf16, name=f"vc{i}") for i in range(2)]
            gT = [wp.tile([128, ROWS], f32, name=f"gT{i}") for i in range(2)]
            sig = [wp.tile([128, ROWS], bf16, name=f"sig{i}") for i in range(2)]
            betaT = wp.tile([2, ROWS], f32, name="betaT")
            betaN = wp.tile([128, 16], f32, name="betaN")
            ogT = [wp.tile([128, ROWS], bf16, name=f"ogT{i}") for i in range(2)]

            # ================= phase 1: projections =================
            with tc.tile_pool(name="ph1", bufs=1) as p1, \
                 tc.tile_pool(name="ph1t", bufs=2) as p1t, \
                 tc.tile_pool(name="ps1", bufs=2, space="PSUM") as ps1:
                xt = p1.tile([128, 16 * ROWS], bf16, name="xt")
                for kt in range(16):
                    nc.sync.dma_start(out=xt[:, kt * ROWS:(kt + 1) * ROWS],
                                      in_=d_xt[kt * 128:(kt + 1) * 128, :])
                wpj = p1.tile([128, 16 * 1026], bf16, name="wpj")
                for kt in range(16):
                    nc.sync.dma_start(out=wpj[:, kt * 1026:(kt + 1) * 1026],
                                      in_=d_wp[kt * 128:(kt + 1) * 128, :])
                wfb = p1.tile([128, 256], bf16, name="wfb")
                nc.sync.dma_start(out=wfb, in_=d_wfb[:, :])
                wgb = p1.tile([128, 256], bf16, name="wgb")
                nc.sync.dma_start(out=wgb, in_=d_wgb[:, :])

                qr = [p1.tile([128, ROWS + 3], bf16, name=f"qr{i}") for i in range(2)]
                kr = [p1.tile([128, ROWS + 3], bf16, name=f"kr{i}") for i in range(2)]
                vr = [p1.tile([128, ROWS + 3], bf16, name=f"vr{i}") for i in range(2)]
                fafT = p1.tile([128, ROWS], bf16, name="fafT")
                fagT = p1.tile([128, ROWS], bf16, name="fagT")
                raws = [qr[0], qr[1], kr[0], kr[1], vr[0], vr[1]]
                for t_ in raws:
                    nc.vector.memset(t_[:, 0:3], 0.0)

                for mc in (6, 7, 8, 0, 2, 4, 1, 3, 5):
                    m0 = mc * 128
                    msz = 2 if mc == 8 else 128
                    for nn in range(ROWS // 512):
                        ps = ps1.tile([128, 512], f32, name="pj")
                        for kt in range(16):
                            nc.tensor.matmul(
                                ps[0:msz, :],
                                lhsT=wpj[:, kt * 1026 + m0: kt * 1026 + m0 + msz],
                                rhs=xt[:, kt * ROWS + nn * 512: kt * ROWS + (nn + 1) * 512],
                                start=(kt == 0), stop=(kt == 15))
                        if mc < 6:
                            nc.scalar.copy(out=raws[mc][:, 3 + nn * 512: 3 + (nn + 1) * 512],
                                           in_=ps[:, :])
                        elif mc == 6:
                            nc.scalar.copy(out=fafT[:, nn * 512:(nn + 1) * 512], in_=ps[:, :])
                        elif mc == 7:
                            nc.scalar.copy(out=fagT[:, nn * 512:(nn + 1) * 512], in_=ps[:, :])
                        else:
                            nc.scalar.activation(out=betaT[:, nn * 512:(nn + 1) * 512],
                                                 in_=ps[0:2, :], func=AF.Sigmoid)

                for c in range(NCH):
                    psb = ps1.tile([128, 512], f32, name="pj")
                    nc.tensor.matmul(psb[0:128, 0:2],
                                     lhsT=betaT[0:2, c * 128:(c + 1) * 128],
                                     rhs=idf32[0:2, 0:2], is_transpose=True)
                    nc.scalar.copy(out=betaN[:, 2 * c:2 * c + 2], in_=psb[:, 0:2])

                for i in range(2):
                    for raw, out_t, cbase in ((qr[i], qc[i], 12 * i),
                                              (kr[i], kc[i], 12 * i + 4),
                                              (vr[i], vc[i], 12 * i + 8)):
                        acc = p1t.tile([128, ROWS], f32, name="cacc")
                        nc.scalar.mul(out=acc, in_=raw[:, 0:ROWS],
                                      mul=cst[:, cbase:cbase + 1])
                        for j in range(1, K):
                            acc2 = p1t.tile([128, ROWS], f32, name="cacc")
                            nc.vector.scalar_tensor_tensor(
                                out=acc2, in0=raw[:, j:j + ROWS],
                                scalar=cst[:, cbase + j:cbase + j + 1],
                                in1=acc, op0=OP.mult, op1=OP.add)
                            acc = acc2
                        sg = p1t.tile([128, ROWS], bf16, name="csig")
                        nc.scalar.activation(out=sg, in_=acc, func=AF.Sigmoid)
                        nc.vector.tensor_tensor(out=out_t, in0=acc, in1=sg, op=OP.mult)

                for i in range(2):
                    for nn in range(ROWS // 512):
                        ps = ps1.tile([128, 512], f32, name="pj")
                        nc.tensor.matmul(ps[:, :], lhsT=wfb[:, 128 * i:128 * (i + 1)],
                                         rhs=fafT[:, nn * 512:(nn + 1) * 512],
                                         start=True, stop=True)
                        tmp2 = p1t.tile([128, 512], f32, name="gtmp2")
                        nc.scalar.activation(out=tmp2, in_=ps[:, :], func=AF.Exp,
                                             bias=cst[:, 24 + i:25 + i])
                        tmp3 = p1t.tile([128, 512], f32, name="gtmp3")
                        nc.scalar.activation(out=tmp3, in_=tmp2, func=AF.Ln, bias=1.0)
                        nc.vector.tensor_scalar(out=gT[i][:, nn * 512:(nn + 1) * 512],
                                                in0=tmp3, scalar1=cst[:, 26 + i:27 + i],
                                                scalar2=None, op0=OP.mult)

                for i in range(2):
                    for hf in range(2):
                        ps = ps1.tile([128, 512], f32, name="pj")
                        for cc in range(4):
                            c = hf * 4 + cc
                            nc.tensor.matmul(ps[:, cc * 128:(cc + 1) * 128],
                                             lhsT=fagT[:, c * 128:(c + 1) * 128],
                                             rhs=wgb[:, 128 * i:128 * (i + 1)],
                                             start=True, stop=True)
                        nc.scalar.activation(out=sig[i][:, hf * 512:(hf + 1) * 512],
                                             in_=ps[:, :], func=AF.Sigmoid)

            # ================= phase 2: recurrence =================
            pwo_cm = tc.tile_pool(name="phwo", bufs=1)
            pwo = pwo_cm.__enter__()
            wo = None
            if with_collective:
                wo = pwo.tile([128, 8 * H], bf16, name="wo")
                for kt in range(8):
                    nc.sync.dma_start(out=wo[:, kt * H:(kt + 1) * H],
                                      in_=d_wo[kt * 128:(kt + 1) * 128, :])
            with tc.tile_pool(name="ph2u", bufs=1) as p2u, \
                 tc.tile_pool(name="ph2", bufs=1) as p2, \
                 tc.tile_pool(name="ph2t", bufs=2) as p2t, \
                 tc.tile_pool(name="ph2s", bufs=3) as p2s, \
                 tc.tile_pool(name="ph2w", bufs=2) as p2w, \
                 tc.tile_pool(name="psGm", bufs=1, space="PSUM") as psGmP, \
                 tc.tile_pool(name="psGx", bufs=2, space="PSUM") as psGxP, \
                 tc.tile_pool(name="psX", bufs=2, space="PSUM") as psXP, \
                 tc.tile_pool(name="psB", bufs=3, space="PSUM") as psBP:
                PH = [{} for _ in range(2)]
                for i in range(2):
                    ph = PH[i]
                    GTz = p2.tile([128, NCH * 129], f32, name="GTz")
                    nc.vector.memset(
                        GTz[:, :].rearrange("p (c z) -> p c z", z=129)[:, :, 0:1], 0.0)
                    for c in range(NCH):
                        nc.vector.tensor_tensor_scan(
                            out=GTz[:, c * 129 + 1:(c + 1) * 129],
                            data0=ones32[:, 0:128],
                            data1=gT[i][:, c * 128:(c + 1) * 128],
                            initial=0.0, op0=OP.mult, op1=OP.add)
                    GT3 = GTz[:, :].rearrange("p (c z) -> p c z", z=129)
                    gview = GT3[:, :, 1:129]
                    zview = GT3[:, :, 0:128]

                    eGT = p2.tile([128, ROWS], bf16, name="eGT")
                    nc.scalar.activation(
                        out=eGT[:, :].rearrange("p (c t) -> p c t", t=128),
                        in_=gview, func=AF.Exp)
                    eGlast = p2.tile([128, NCH], f32, name=f"eGl{i}")
                    nc.scalar.activation(
                        out=eGlast[:, :].rearrange("p (c o) -> p c o", o=1),
                        in_=GT3[:, :, 128:129], func=AF.Exp)
                    ph["eGlast"] = eGlast

                    kn = p2.tile([128, ROWS], bf16, name="kn")
                    sq = p2u.tile([128, ROWS], f32, name="sq")
                    nc.scalar.activation(out=sq, in_=kc[i], func=AF.Square)
                    rrow = p2t.tile([1, ROWS], f32, name="rrow")
                    for hf in range(2):
                        psq = psGxP.tile([128, 512], f32, name="psGx")
                        nc.tensor.matmul(psq[0:1, :], lhsT=ones32[:, 0:1],
                                         rhs=sq[:, hf * 512:(hf + 1) * 512],
                                         start=True, stop=True)
                        nc.scalar.activation(out=rrow[0:1, hf * 512:(hf + 1) * 512],
                                             in_=psq[0:1, :], func=AF.Sqrt,
                                             bias=cst[0:1, 28:29])
                    nc.vector.reciprocal(out=rrow, in_=rrow)
                    for hf in range(2):
                        psbc = psGxP.tile([128, 512], f32, name="psGx")
                        for cc in range(4):
                            c = hf * 4 + cc
                            nc.tensor.matmul(psbc[:, cc * 128:(cc + 1) * 128],
                                             lhsT=ones32[0:1, 0:128],
                                             rhs=rrow[0:1, c * 128:(c + 1) * 128],
                                             start=True, stop=True)
                        nc.vector.tensor_tensor(out=kn[:, hf * 512:(hf + 1) * 512],
                                                in0=kc[i][:, hf * 512:(hf + 1) * 512],
                                                in1=psbc, op=OP.mult)
                    sqq = p2u.tile([128, ROWS], f32, name="sq")
                    nc.scalar.activation(out=sqq, in_=qc[i], func=AF.Square)
                    psrq = psBP.tile([128, 128], f32, name="psB")
                    for c in range(NCH):
                        nc.tensor.matmul(psrq[:, c:c + 1],
                                         lhsT=sqq[:, c * 128:(c + 1) * 128],
                                         rhs=ones32[:, 0:1],
                                         start=True, stop=True)
                    rqn = p2.tile([128, NCH], f32, name=f"rqn{i}")
                    nc.scalar.activation(out=rqn, in_=psrq[:, 0:NCH], func=AF.Sqrt,
                                         scale=float(D), bias=cst[:, 29:30])
                    nc.vector.reciprocal(out=rqn, in_=rqn)
                    ph["rqn"] = rqn
                    qs = qc[i]

                    KeGT = p2.tile([128, ROWS], bf16, name=f"KeGT{i}")
                    nc.vector.tensor_tensor(out=KeGT, in0=kn, in1=eGT, op=OP.mult)
                    QeGsT = p2.tile([128, ROWS], bf16, name=f"QeG{i}")
                    nc.vector.tensor_tensor(out=QeGsT, in0=qs, in1=eGT, op=OP.mult)
                    qk = p2.tile([128, ROWS], bf16, name="qk")
                    nc.vector.tensor_tensor(out=qk, in0=qs, in1=kn, op=OP.mult)
                    ph["KeGT"] = KeGT
                    ph["QeGsT"] = QeGsT

                    etT = p2t.tile([128, ROWS], bf16, name="etT")
                    for c in range(NCH):
                        nc.scalar.activation(out=etT[:, c * 128:(c + 1) * 128],
                                             in_=GTz[:, c * 129 + 1:(c + 1) * 129],
                                             func=AF.Exp, scale=-1.0,
                                             bias=GTz[:, c * 129 + 128:c * 129 + 129])
                    KetT = p2t.tile([128, ROWS], bf16, name="KetT")
                    nc.vector.tensor_tensor(out=KetT, in0=kn, in1=etT, op=OP.mult)
                    KetN = p2.tile([128, ROWS], bf16, name=f"KetN{i}")
                    VN = p2.tile([128, ROWS], bf16, name=f"VN{i}")
                    ph["KetN"] = KetN
                    ph["VN"] = VN
                    for c in range(NCH):
                        nc.sync.dma_start_transpose(
                            out=KetN[:, c * 128:(c + 1) * 128],
                            in_=KetT[:, c * 128:(c + 1) * 128])
                        nc.sync.dma_start_transpose(
                            out=VN[:, c * 128:(c + 1) * 128],
                            in_=vc[i][:, c * 128:(c + 1) * 128])

                    lvl_ops = []
                    for li, bs in enumerate((128, 64, 32, 16, 8)):
                        nb = 128 // bs
                        cen_off = bs // 2 if bs > 8 else 0
                        g4 = gview.rearrange("p c (nb bs) -> p c nb bs", bs=bs)
                        z4 = zview.rearrange("p c (nb bs) -> p c nb bs", bs=bs)
                        cen = z4[:, :, :, cen_off:cen_off + 1].broadcast_to(
                            [128, NCH, nb, bs])
                        dlt = p2u.tile([128, ROWS], f32, name="dlt")
                        nc.vector.tensor_tensor(
                            out=dlt[:, :].rearrange("p (c nb bs) -> p c nb bs",
                                                    c=NCH, bs=bs),
                            in0=g4, in1=cen, op=OP.subtract)
                        el = p2u.tile([128, ROWS], bf16, name="el")
                        er = p2u.tile([128, ROWS], bf16, name="er")
                        KEL = p2.tile([128, ROWS], bf16, name=f"KEL{li}")
                        KER = p2.tile([128, ROWS], bf16, name=f"KER{li}")
                        QEL = p2.tile([128, ROWS], bf16, name=f"QEL{li}")

                        def hv(t_, lohi):
                            v = t_[:, :].rearrange("p (c nb b2) -> p c nb b2",
                                                   c=NCH, b2=bs)
                            return (v[:, :, :, bs // 2:bs] if lohi else
                                    v[:, :, :, 0:bs // 2])

                        if li <= 1:
                            nc.scalar.activation(out=hv(el, 1), in_=hv(dlt, 1),
                                                 func=AF.Exp)
                            nc.scalar.activation(out=hv(er, 0), in_=hv(dlt, 0),
                                                 func=AF.Exp, scale=-1.0)
                            nc.vector.tensor_tensor(out=hv(KEL, 1), in0=hv(kn, 1),
                                                    in1=hv(el, 1), op=OP.mult)
                            nc.vector.tensor_tensor(out=hv(KER, 0), in0=hv(kn, 0),
                                                    in1=hv(er, 0), op=OP.mult)
                            nc.vector.tensor_tensor(out=hv(QEL, 1), in0=hv(qs, 1),
                                                    in1=hv(el, 1), op=OP.mult)
                        elif bs > 8:
                            nc.vector.memset(hv(KEL, 0), 0.0)
                            nc.vector.memset(hv(QEL, 0), 0.0)
                            nc.scalar.activation(out=hv(el, 1), in_=hv(dlt, 1),
                                                 func=AF.Exp)
                            nc.scalar.activation(out=er, in_=dlt, func=AF.Exp,
                                                 scale=-1.0)
                            nc.vector.tensor_tensor(out=hv(KEL, 1), in0=hv(kn, 1),
                                                    in1=hv(el, 1), op=OP.mult)
                            nc.vector.tensor_tensor(out=KER, in0=kn, in1=er,
                                                    op=OP.mult)
                            nc.vector.tensor_tensor(out=hv(QEL, 1), in0=hv(qs, 1),
                                                    in1=hv(el, 1), op=OP.mult)
                        else:
                            nc.scalar.activation(out=el, in_=dlt, func=AF.Exp)
                            tmx = p2u.tile([128, ROWS], f32, name="tmx")
                            nc.vector.tensor_scalar(out=tmx, in0=dlt, scalar1=-85.0,
                                                    scalar2=None, op0=OP.max)
                            nc.scalar.activation(out=er, in_=tmx, func=AF.Exp,
                                                 scale=-1.0)
                            nc.vector.tensor_tensor(out=KEL, in0=kn, in1=el, op=OP.mult)
                            nc.vector.tensor_tensor(out=KER, in0=kn, in1=er, op=OP.mult)
                            nc.vector.tensor_tensor(out=QEL, in0=qs, in1=el, op=OP.mult)
                        lvl_ops.append((KEL, KER, QEL))

                    TmT = [p2.tile([128, 128], bf16, name=f"TmT{i}_{c}")
                           for c in range(NCH)]
                    AqT = [p2.tile([128, 128], bf16, name=f"AqT{i}_{c}")
                           for c in range(NCH)]
                    ph["TmT"] = TmT
                    ph["AqT"] = AqT
                    Nb = p2.tile([128, ROWS], bf16, name="Nb")
                    KEL0, KER0, QEL0 = lvl_ops[0]
                    KEL1, KER1, QEL1 = lvl_ops[1]
                    KEL2, KER2, QEL2 = lvl_ops[2]
                    KEL3, KER3, QEL3 = lvl_ops[3]
                    KEL8, KER8, QEL8 = lvl_ops[4]
                    for half in range(2):
                        for orient in range(2):
                            psMain = psGmP.tile([128, 512], f32, name="psGm")
                            nc.vector.memset(psMain, 0.0)
                            psLs = []
                            for lvl_idx, (KA, KB) in enumerate((
                                    (KER2, KEL2) if orient == 0 else (KER2, QEL2),
                                    (KER3, KEL3) if orient == 0 else (KER3, QEL3),
                                    (KER8, KEL8) if orient == 0 else (KER8, QEL8))):
                                psL = psGxP.tile([128, 512], f32, name="psGx")
                                for cc in range(4):
                                    c = half * 4 + cc
                                    sl = slice(c * 128, (c + 1) * 128)
                                    nc.tensor.matmul(
                                        psL[:, cc * 128:(cc + 1) * 128],
                                        lhsT=KA[:, sl], rhs=KB[:, sl],
                                        start=True, stop=True)
                                psLs.append(psL)
                            RL0 = KEL0 if orient == 0 else QEL0
                            RL1 = KEL1 if orient == 0 else QEL1
                            for cc in range(4):
                                c = half * 4 + cc
                                o0 = cc * 128
                                nc.tensor.matmul(
                                    psMain[0:64, o0 + 64:o0 + 128],
                                    lhsT=KER0[:, c * 128:c * 128 + 64],
                                    rhs=RL0[:, c * 128 + 64:c * 128 + 128],
                                    start=True, stop=True)
                                nc.tensor.matmul(
                                    psMain[0:32, o0 + 32:o0 + 64],
                                    lhsT=KER1[:, c * 128:c * 128 + 32],
                                    rhs=RL1[:, c * 128 + 32:c * 128 + 64],
                                    start=True, stop=True)
                                nc.tensor.matmul(
                                    psMain[64:96, o0 + 96:o0 + 128],
                                    lhsT=KER1[:, c * 128 + 64:c * 128 + 96],
                                    rhs=RL1[:, c * 128 + 96:c * 128 + 128],
                                    start=True, stop=True)
                            a0 = p2w.tile([128, 512], f32, name="a0")
                            nc.scalar.copy(out=a0, in_=psMain)
                            a1 = p2w.tile([128, 512], f32, name="a1")
                            nc.vector.select(out=a1, mask=masksb["m32Tx4"],
                                             on_true=psLs[0], on_false=a0)
                            a3 = p2w.tile([128, 512], f32, name="a1")
                            nc.vector.select(out=a3, mask=masksb["m16Tx4"],
                                             on_true=psLs[1], on_false=a1)
                            gsum = p2w.tile([128, 512], f32, name="gsum")
                            nc.vector.select(out=gsum, mask=masksb["m8Tx4"],
                                             on_true=psLs[2], on_false=a3)
                            for cc in range(4):
                                c = half * 4 + cc
                                sl = slice(c * 128, (c + 1) * 128)
                                gsl = slice(cc * 128, (cc + 1) * 128)
                                if orient == 0:
                                    nc.scalar.mul(
                                        out=Nb[:, c * 128:(c + 1) * 128],
                                        in_=gsum[:, gsl],
                                        mul=betaN[:, 2 * c + i:2 * c + i + 1])
                                else:
                                    psd = psBP.tile([128, 128], f32, name="psB")
                                    nc.tensor.matmul(psd[:, 0:1], lhsT=qk[:, sl],
                                                     rhs=onesbf[:, 0:1],
                                                     start=True, stop=True)
                                    tmpA = p2s.tile([128, 128], bf16, name="tmpA")
                                    nc.scalar.copy(out=tmpA, in_=gsum[:, gsl])
                                    nc.vector.scalar_tensor_tensor(
                                        out=AqT[c], in0=idbf, scalar=psd[:, 0:1],
                                        in1=tmpA, op0=OP.mult, op1=OP.add)

                    idx3 = idbf[:, :].unsqueeze(1).broadcast_to([128, 4, 128])
                    for hf in range(2):
                        xprev = [idbf] * 4
                        for j in range(JH):
                            psXb = psXP.tile([128, 512], f32, name="psX")
                            for cc in range(4):
                                c = hf * 4 + cc
                                nc.tensor.matmul(psXb[:, cc * 128:(cc + 1) * 128],
                                                 lhsT=Nb[:, c * 128:(c + 1) * 128],
                                                 rhs=xprev[cc], start=True, stop=True)
                            xnb = p2s.tile([128, 512], bf16, name="xh")
                            nc.vector.scalar_tensor_tensor(
                                out=xnb[:, :].rearrange("p (c t) -> p c t", t=128),
                                in0=idx3, scalar=1.0,
                                in1=psXb[:, :].rearrange("p (c t) -> p c t", t=128),
                                op0=OP.mult, op1=OP.subtract)
                            xprev = [xnb[:, cc * 128:(cc + 1) * 128] for cc in range(4)]
                        for cc in range(4):
                            c = hf * 4 + cc
                            nc.sync.dma_start_transpose(out=TmT[c][:, :], in_=xprev[cc])

                    ogg = p2t.tile([128, ROWS], f32, name=f"ogg{i}")
                    ph["ogg"] = ogg
                    Sf = p2.tile([128, 128], f32, name=f"Sf{i}")
                    Sb = p2.tile([128, 128], bf16, name=f"Sb{i}")
                    nc.vector.memset(Sf, 0.0)
                    nc.vector.memset(Sb, 0.0)
                    ph["Sf"] = Sf
                    ph["Sb"] = Sb
                # serial chain for this head
                    for c in range(NCH):
                        sl = slice(c * 128, (c + 1) * 128)
                        Sf, Sb = ph["Sf"], ph["Sb"]
                        psKS = psBP.tile([128, 128], f32, name="psB")
                        nc.tensor.matmul(psKS[:, :], lhsT=ph["KeGT"][:, sl], rhs=Sb,
                                         start=True, stop=True)
                        Rr = p2s.tile([128, 128], bf16, name="Rr")
                        nc.vector.tensor_tensor(out=Rr, in0=ph["VN"][:, sl], in1=psKS,
                                                op=OP.subtract)
                        psY = psBP.tile([128, 128], f32, name="psB")
                        nc.tensor.matmul(psY[:, :], lhsT=ph["TmT"][c], rhs=Rr,
                                         start=True, stop=True)
                        Dl = p2s.tile([128, 128], bf16, name="Dl")
                        nc.scalar.mul(out=Dl, in_=psY,
                                      mul=betaN[:, 2 * c + i:2 * c + i + 1])
                        psO = psBP.tile([128, 128], f32, name="psB")
                        nc.tensor.matmul(psO[:, :], lhsT=ph["QeGsT"][:, sl], rhs=Sb,
                                         start=True, stop=False)
                        nc.tensor.matmul(psO[:, :], lhsT=ph["AqT"][c], rhs=Dl,
                                         start=False, stop=True)
                        psS = psBP.tile([128, 128], f32, name="psB")
                        nc.tensor.matmul(psS[:, :], lhsT=ph["KetN"][:, sl], rhs=Dl,
                                         start=True, stop=True)
                        nc.vector.scalar_tensor_tensor(
                            out=Sf, in0=Sf, scalar=ph["eGlast"][:, c:c + 1],
                            in1=psS, op0=OP.mult, op1=OP.add)
                        nc.scalar.copy(out=Sb, in_=Sf)
                        nc.vector.scalar_tensor_tensor(
                            out=ph["ogg"][:, sl], in0=psO,
                            scalar=ph["rqn"][:, c:c + 1],
                            in1=sig[i][:, sl], op0=OP.mult, op1=OP.mult)

                # batched gated-RMS + transpose per head
                    ogg = ph["ogg"]
                    sqo = p2u.tile([128, ROWS], f32, name="sqo")
                    nc.scalar.activation(out=sqo, in_=ogg, func=AF.Square)
                    ssr = p2s.tile([128, NCH], f32, name="ssr")
                    nc.vector.tensor_reduce(
                        out=ssr, in_=sqo[:, :].rearrange("p (c t) -> p c t", t=128),
                        axis=mybir.AxisListType.X, op=OP.add)
                    rmsr = p2s.tile([128, NCH], f32, name="rmsr")
                    nc.scalar.activation(out=rmsr, in_=ssr, func=AF.Sqrt,
                                         scale=1.0 / D, bias=cst[:, 28:29])
                    nc.vector.reciprocal(out=rmsr, in_=rmsr)
                    ogn = p2u.tile([128, ROWS], bf16, name="ogn")
                    nc.vector.tensor_tensor(
                        out=ogn[:, :].rearrange("p (c t) -> p c t", t=128),
                        in0=ogg[:, :].rearrange("p (c t) -> p c t", t=128),
                        in1=rmsr[:, :].unsqueeze(2).broadcast_to([128, NCH, 128]),
                        op=OP.mult)
                    for c in range(NCH):
                        sl = slice(c * 128, (c + 1) * 128)
                        nc.sync.dma_start_transpose(out=ogT[i][:, sl], in_=ogn[:, sl])


            # ================= phase 3: A2A + o_proj =================
            if debug_ogt:
                with tc.tile_pool(name="dbg", bufs=1) as dbp:
                    for i in range(2):
                        tmp = dbp.tile([128, ROWS], f32, name="dbgcp")
                        nc.scalar.copy(out=tmp, in_=ogT[i])
                        nc.sync.dma_start(out=d_ogt_dbg[i * 128:(i + 1) * 128, :], in_=tmp)

            if with_collective:
                with tc.tile_pool(name="ph3", bufs=1) as p3, \
                     tc.tile_pool(name="ph3t", bufs=2) as p3t, \
                     tc.tile_pool(name="ps3", bufs=2, space="PSUM") as ps3:
                    # shard j (rows 256j..256j+256) = [h0 | h1] cols 128j..128j+128
                    for j in range(8):
                        for i in range(2):
                            nc.sync.dma_start(
                                out=d_cin[256 * j + 128 * i:256 * j + 128 * (i + 1), :],
                                in_=ogT[i][:, 128 * j:128 * (j + 1)])
                    nc.gpsimd.collective_compute(
                        "AllToAll", mybir.AluOpType.bypass,
                        ins=[d_cin[:, :]], outs=[d_cout[:, :]],
                        replica_groups=[list(range(8))])
                    # cout rows [0:1024) = batch-0 og (heads 0..7), [1024:2048) batch-1
                    oga = p3.tile([128, 16 * 128], bf16, name="oga")
                    for kt in range(16):
                        nc.sync.dma_start(out=oga[:, kt * 128:(kt + 1) * 128],
                                          in_=d_cout[kt * 128:(kt + 1) * 128, :])
                    for b2 in range(2):
                        for nn2 in range(H // 512):
                            pso = ps3.tile([128, 512], f32, name="pso")
                            for kt in range(8):
                                nc.tensor.matmul(
                                    pso[:, :],
                                    lhsT=oga[:, (b2 * 8 + kt) * 128:
                                             (b2 * 8 + kt + 1) * 128],
                                    rhs=wo[:, kt * H + nn2 * 512:
                                           kt * H + (nn2 + 1) * 512],
                                    start=(kt == 0), stop=(kt == 7))
                            osb = p3t.tile([128, 512], f32, name="osb")
                            nc.scalar.copy(out=osb, in_=pso)
                            nc.sync.dma_start(
                                out=d_out[b2 * 128:(b2 + 1) * 128,
                                          nn2 * 512:(nn2 + 1) * 512],
                                in_=osb)

            pwo_cm.__exit__(None, None, None)

    if split_waits:
        _split_excess_waits(nc, mybir)
    return nc


def _host_inputs(inputs):
    x = np.asarray(inputs['hidden_states'], np.float32)
    Wq, Wk, Wv = (np.asarray(inputs[k], np.float32) for k in ('Wq', 'Wk', 'Wv'))
    conv_q, conv_k, conv_v = (np.asarray(inputs[k], np.float32)
                              for k in ('conv_q', 'conv_k', 'conv_v'))
    A_log = np.asarray(inputs['A_log'], np.float32)
    dt_bias = np.asarray(inputs['dt_bias'], np.float32)
    Wfa, Wfb = np.asarray(inputs['Wfa'], np.float32), np.asarray(inputs['Wfb'], np.float32)
    Wb = np.asarray(inputs['Wb'], np.float32)
    Wga, Wgb = np.asarray(inputs['Wga'], np.float32), np.asarray(inputs['Wgb'], np.float32)
    nw = np.asarray(inputs['norm_weight'], np.float32)
    Wo = np.asarray(inputs['Wo'], np.float32)
    WoT_folded = np.ascontiguousarray((Wo * np.tile(nw, NH)[None, :]).T)

    in_maps = []
    for cidx in range(NCORES):
        b, hp = cidx // 4, cidx % 4
        heads = (2 * hp, 2 * hp + 1)
        xT = np.ascontiguousarray(x[b].T)
        Wsl = []
        for h in heads: Wsl.append(Wq[128 * h:128 * (h + 1)])
        for h in heads: Wsl.append(Wk[128 * h:128 * (h + 1)])
        for h in heads: Wsl.append(Wv[128 * h:128 * (h + 1)])
        Wsl.append(Wfa); Wsl.append(Wga)
        Wsl.append(Wb[list(heads)])
        WprojT = np.ascontiguousarray(np.concatenate(Wsl, axis=0).T)
        consts = np.zeros((128, 32), np.float32)
        for i, h in enumerate(heads):
            consts[:, 12 * i + 0:12 * i + 4] = conv_q[128 * h:128 * (h + 1)]
            consts[:, 12 * i + 4:12 * i + 8] = conv_k[128 * h:128 * (h + 1)]
            consts[:, 12 * i + 8:12 * i + 12] = conv_v[128 * h:128 * (h + 1)]
            consts[:, 24 + i] = dt_bias[128 * h:128 * (h + 1)]
            consts[:, 26 + i] = -np.exp(A_log[h])
        consts[:, 28] = EPS
        consts[:, 29] = D * EPS
        wfb2 = np.concatenate([Wfb[128 * h:128 * (h + 1)].T for h in heads], axis=1)
        wgb2 = np.concatenate([Wgb[128 * h:128 * (h + 1)].T for h in heads], axis=1)
        in_maps.append({
            "xt": xT.astype(BF16),
            "wp": WprojT.astype(BF16),
            "wfb": np.ascontiguousarray(wfb2).astype(BF16),
            "wgb": np.ascontiguousarray(wgb2).astype(BF16),
            "wo": WoT_folded.astype(BF16),
            "cst": consts,
        })
    return in_maps


def _estimate_exec_ns(nc):
    """Best-effort single-core HW time estimate from the instruction cost
    model (NTFF profiling is unavailable under the axon client)."""
    try:
        from concourse.timeline_sim import TimelineSim
        return int(TimelineSim(nc, trace=False).simulate())
    except Exception:
        return None


def kernel(**inputs):
    global LAST_EXEC_NS
    from concourse.bass_utils import run_bass_kernel_spmd

    if "nc" not in _CACHE:
        _CACHE["nc"] = build_nc()
    nc = _CACHE["nc"]
    in_maps = _host_inputs(inputs)
    res = run_bass_kernel_spmd(nc, in_maps, core_ids=list(range(NCORES)), trace=False)
    if res.exec_time_ns is not None:
        LAST_EXEC_NS = res.exec_time_ns
    else:
        LAST_EXEC_NS = _estimate_exec_ns(nc)
    out = np.zeros((B * T, H), np.float32)
    for cidx in range(NCORES):
        r = res.results[cidx]["out"]
        out[128 * cidx:128 * (cidx + 1)] = r[0:128]
        out[T + 128 * cidx:T + 128 * (cidx + 1)] = r[128:256]
    return out



# revision 4
# speedup vs baseline: 1.0608x; 1.0117x over previous
"""nn_KimiDecoderLayer on 8 TRN2 NeuronCores, fully on-device.

Sharding: TP4 over heads x DP2 over batch. Core c in 0..3: batch 0, heads
(2c, 2c+1); core c+4: batch 1, same heads. Projections, short conv, gated
delta-rule recurrence (chunked, exact hierarchical decay factorization
matching the reference), gated RMS norm run head-local; o_proj runs
row-parallel after an in-kernel AllToAll of the normed outputs.
"""
import numpy as np
import ml_dtypes

B, T, H = 2, 1024, 2048
NH, D = 8, 128
P = NH * D
K = 4
EPS = 1e-6
SCALE = D ** -0.5
C = 128
JH = 4
NCORES = 8
ROWS = 1024          # rows per core (one batch)
NCH = 8              # chunks per head

LAST_EXEC_NS = None
_CACHE = {}

BF16 = ml_dtypes.bfloat16


def _split_excess_waits(nc, mybir, max_waits=1):
    """walrus in this env rejects >1 sem-wait per instruction; hoist extras
    onto preceding same-engine NoOps (semantically identical)."""
    n = 0
    for f in nc.m.functions:
        for blk in f.blocks:
            new = []
            changed = False
            for inst in blk.instructions:
                si = inst.sync_info
                if si is not None and si.on_wait is not None and len(si.on_wait) > max_waits:
                    waits = list(si.on_wait)
                    groups = [waits[i:i + max_waits] for i in range(0, len(waits), max_waits)]
                    for gi, g in enumerate(groups[:-1]):
                        new.append(mybir.InstNoOp(
                            name=f"{inst.name}-ws{gi}", engine=inst.engine,
                            sync_info=mybir.SyncInfo(on_wait=g, on_update=[]),
                            bass_nofuse=True))
                    inst.sync_info = mybir.SyncInfo(on_wait=groups[-1],
                                                    on_update=list(si.on_update))
                    changed = True
                    n += 1
                new.append(inst)
            if changed:
                blk.instructions = new
    return n


def _masks():
    def rect(bs):
        m = np.zeros((C, C), np.float32)
        for b0 in range(0, C, bs):
            m[b0 + bs // 2:b0 + bs, b0:b0 + bs // 2] = 1
        return m
    m32, m16 = rect(32), rect(16)
    mask8 = np.zeros((C, C), np.float32)
    for b0 in range(0, C, 8):
        mask8[b0:b0 + 8, b0:b0 + 8] = 1
    m8 = np.tril(mask8, -1)
    return m32, m16, m8


def build_nc(with_collective=True, debug_ogt=False, split_waits=True):
    import concourse.bass as bass
    import concourse.tile as tile
    from concourse import mybir

    f32 = mybir.dt.float32
    bf16 = mybir.dt.bfloat16

    nc = bass.Bass(num_devices=NCORES)
    d_xt = nc.dram_tensor("xt", (H, ROWS), bf16, kind="ExternalInput")
    d_wp = nc.dram_tensor("wp", (H, 1026), bf16, kind="ExternalInput")
    d_wfb = nc.dram_tensor("wfb", (128, 256), bf16, kind="ExternalInput")
    d_wgb = nc.dram_tensor("wgb", (128, 256), bf16, kind="ExternalInput")
    d_wo = nc.dram_tensor("wo", (P, H), bf16, kind="ExternalInput")
    d_cst = nc.dram_tensor("cst", (128, 32), f32, kind="ExternalInput")
    d_out = nc.dram_tensor("out", (256, H), f32, kind="ExternalOutput")
    d_ogt_dbg = None
    if debug_ogt:
        d_ogt_dbg = nc.dram_tensor("ogt_dbg", (256, ROWS), f32, kind="ExternalOutput")
    if with_collective:
        d_cin = nc.dram_tensor("a2a_in", (2 * P, 128), bf16, kind="Internal")
        d_cout = nc.dram_tensor("a2a_out", (2 * P, 128), bf16, kind="Internal")

    m32_np, m16_np, m8_np = _masks()
    d_m = {}
    d_mnp = {"m32": m32_np, "m16": m16_np, "m8": m8_np,
             "m32T": np.ascontiguousarray(m32_np.T),
             "m16T": np.ascontiguousarray(m16_np.T),
             "m8T": np.ascontiguousarray(m8_np.T)}
    for nm, arr in d_mnp.items():
        d_m[nm] = nc.inline_tensor(arr.astype(np.float32), name=nm)
    d_m4 = {}
    for nm in ("m32T", "m16T", "m8T"):
        arr4 = np.tile(d_mnp[nm], (1, 4))
        d_m4[nm] = nc.inline_tensor(arr4.astype(np.uint8), name=nm + "x4")
    d_id_bf = nc.inline_tensor(np.eye(128, dtype=BF16), name="idbf")
    d_id_f32 = nc.inline_tensor(np.eye(128, dtype=np.float32), name="idf32")
    d_ones_f32 = nc.inline_tensor(np.ones((128, 128), np.float32), name="ones32")
    d_ones_bf = nc.inline_tensor(np.ones((128, 1), BF16), name="onesbf")

    AF = mybir.ActivationFunctionType
    OP = mybir.AluOpType

    with tile.TileContext(nc) as tc:
        with tc.tile_pool(name="wpool", bufs=1) as wp:
            masksb = {}
            u8 = mybir.dt.uint8
            for nm in ("m32T", "m16T", "m8T"):
                mt = wp.tile([128, 512], u8, name=nm + "x4")
                nc.sync.dma_start(out=mt, in_=d_m4[nm][:, :])
                masksb[nm + "x4"] = mt
            idbf = wp.tile([128, 128], bf16, name="idbf")
            nc.sync.dma_start(out=idbf, in_=d_id_bf[:, :])
            idf32 = wp.tile([128, 128], f32, name="idf32")
            nc.sync.dma_start(out=idf32, in_=d_id_f32[:, :])
            ones32 = wp.tile([128, 128], f32, name="ones32")
            nc.sync.dma_start(out=ones32, in_=d_ones_f32[:, :])
            onesbf = wp.tile([128, 1], bf16, name="onesbf")
            nc.sync.dma_start(out=onesbf, in_=d_ones_bf[:, :])
            cst = wp.tile([128, 32], f32, name="cst")
            nc.sync.dma_start(out=cst, in_=d_cst[:, :])

            qc = [wp.tile([128, ROWS], bf16, name=f"qc{i}") for i in range(2)]
            kc = [wp.tile([128, ROWS], bf16, name=f"kc{i}") for i in range(2)]
            vc = [wp.tile([128, ROWS], bf16, name=f"vc{i}") for i in range(2)]
            gT = [wp.tile([128, ROWS], f32, name=f"gT{i}") for i in range(2)]
            sig = [wp.tile([128, ROWS], bf16, name=f"sig{i}") for i in range(2)]
            betaT = wp.tile([2, ROWS], f32, name="betaT")
            betaN = wp.tile([128, 16], f32, name="betaN")
            ogT = [wp.tile([128, ROWS], bf16, name=f"ogT{i}") for i in range(2)]

            # ================= phase 1: projections =================
            with tc.tile_pool(name="ph1", bufs=1) as p1, \
                 tc.tile_pool(name="ph1t", bufs=2) as p1t, \
                 tc.tile_pool(name="ps1", bufs=2, space="PSUM") as ps1:
                xt = p1.tile([128, 16 * ROWS], bf16, name="xt")
                for kt in range(16):
                    nc.sync.dma_start(out=xt[:, kt * ROWS:(kt + 1) * ROWS],
                                      in_=d_xt[kt * 128:(kt + 1) * 128, :])
                wpj = p1.tile([128, 16 * 1026], bf16, name="wpj")
                for kt in range(16):
                    nc.sync.dma_start(out=wpj[:, kt * 1026:(kt + 1) * 1026],
                                      in_=d_wp[kt * 128:(kt + 1) * 128, :])
                wfb = p1.tile([128, 256], bf16, name="wfb")
                nc.sync.dma_start(out=wfb, in_=d_wfb[:, :])
                wgb = p1.tile([128, 256], bf16, name="wgb")
                nc.sync.dma_start(out=wgb, in_=d_wgb[:, :])

                qr = [p1.tile([128, ROWS + 3], bf16, name=f"qr{i}") for i in range(2)]
                kr = [p1.tile([128, ROWS + 3], bf16, name=f"kr{i}") for i in range(2)]
                vr = [p1.tile([128, ROWS + 3], bf16, name=f"vr{i}") for i in range(2)]
                fafT = p1.tile([128, ROWS], bf16, name="fafT")
                fagT = p1.tile([128, ROWS], bf16, name="fagT")
                raws = [qr[0], qr[1], kr[0], kr[1], vr[0], vr[1]]
                for t_ in raws:
                    nc.vector.memset(t_[:, 0:3], 0.0)

                for mc in (6, 7, 8, 0, 2, 4, 1, 3, 5):
                    m0 = mc * 128
                    msz = 2 if mc == 8 else 128
                    for nn in range(ROWS // 512):
                        ps = ps1.tile([128, 512], f32, name="pj")
                        for kt in range(16):
                            nc.tensor.matmul(
                                ps[0:msz, :],
                                lhsT=wpj[:, kt * 1026 + m0: kt * 1026 + m0 + msz],
                                rhs=xt[:, kt * ROWS + nn * 512: kt * ROWS + (nn + 1) * 512],
                                start=(kt == 0), stop=(kt == 15))
                        if mc < 6:
                            nc.scalar.copy(out=raws[mc][:, 3 + nn * 512: 3 + (nn + 1) * 512],
                                           in_=ps[:, :])
                        elif mc == 6:
                            nc.scalar.copy(out=fafT[:, nn * 512:(nn + 1) * 512], in_=ps[:, :])
                        elif mc == 7:
                            nc.scalar.copy(out=fagT[:, nn * 512:(nn + 1) * 512], in_=ps[:, :])
                        else:
                            nc.scalar.activation(out=betaT[:, nn * 512:(nn + 1) * 512],
                                                 in_=ps[0:2, :], func=AF.Sigmoid)

                for c in range(NCH):
                    psb = ps1.tile([128, 512], f32, name="pj")
                    nc.tensor.matmul(psb[0:128, 0:2],
                                     lhsT=betaT[0:2, c * 128:(c + 1) * 128],
                                     rhs=idf32[0:2, 0:2], is_transpose=True)
                    nc.scalar.copy(out=betaN[:, 2 * c:2 * c + 2], in_=psb[:, 0:2])

                for i in range(2):
                    for raw, out_t, cbase in ((qr[i], qc[i], 12 * i),
                                              (kr[i], kc[i], 12 * i + 4),
                                              (vr[i], vc[i], 12 * i + 8)):
                        acc = p1t.tile([128, ROWS], f32, name="cacc")
                        nc.scalar.mul(out=acc, in_=raw[:, 0:ROWS],
                                      mul=cst[:, cbase:cbase + 1])
                        for j in range(1, K):
                            acc2 = p1t.tile([128, ROWS], f32, name="cacc")
                            nc.vector.scalar_tensor_tensor(
                                out=acc2, in0=raw[:, j:j + ROWS],
                                scalar=cst[:, cbase + j:cbase + j + 1],
                                in1=acc, op0=OP.mult, op1=OP.add)
                            acc = acc2
                        sg = p1t.tile([128, ROWS], bf16, name="csig")
                        nc.scalar.activation(out=sg, in_=acc, func=AF.Sigmoid)
                        nc.vector.tensor_tensor(out=out_t, in0=acc, in1=sg, op=OP.mult)

                for i in range(2):
                    for nn in range(ROWS // 512):
                        ps = ps1.tile([128, 512], f32, name="pj")
                        nc.tensor.matmul(ps[:, :], lhsT=wfb[:, 128 * i:128 * (i + 1)],
                                         rhs=fafT[:, nn * 512:(nn + 1) * 512],
                                         start=True, stop=True)
                        tmp2 = p1t.tile([128, 512], f32, name="gtmp2")
                        nc.scalar.activation(out=tmp2, in_=ps[:, :], func=AF.Exp,
                                             bias=cst[:, 24 + i:25 + i])
                        tmp3 = p1t.tile([128, 512], f32, name="gtmp3")
                        nc.scalar.activation(out=tmp3, in_=tmp2, func=AF.Ln, bias=1.0)
                        nc.vector.tensor_scalar(out=gT[i][:, nn * 512:(nn + 1) * 512],
                                                in0=tmp3, scalar1=cst[:, 26 + i:27 + i],
                                                scalar2=None, op0=OP.mult)

                for i in range(2):
                    for hf in range(2):
                        ps = ps1.tile([128, 512], f32, name="pj")
                        for cc in range(4):
                            c = hf * 4 + cc
                            nc.tensor.matmul(ps[:, cc * 128:(cc + 1) * 128],
                                             lhsT=fagT[:, c * 128:(c + 1) * 128],
                                             rhs=wgb[:, 128 * i:128 * (i + 1)],
                                             start=True, stop=True)
                        nc.scalar.activation(out=sig[i][:, hf * 512:(hf + 1) * 512],
                                             in_=ps[:, :], func=AF.Sigmoid)

            # ================= phase 2: recurrence =================
            pwo_cm = tc.tile_pool(name="phwo", bufs=1)
            pwo = pwo_cm.__enter__()
            wo = None
            if with_collective:
                wo = pwo.tile([128, 8 * H], bf16, name="wo")
                for kt in range(8):
                    nc.sync.dma_start(out=wo[:, kt * H:(kt + 1) * H],
                                      in_=d_wo[kt * 128:(kt + 1) * 128, :])
            with tc.tile_pool(name="ph2u", bufs=1) as p2u, \
                 tc.tile_pool(name="ph2", bufs=1) as p2, \
                 tc.tile_pool(name="ph2t", bufs=2) as p2t, \
                 tc.tile_pool(name="ph2s", bufs=3) as p2s, \
                 tc.tile_pool(name="ph2w", bufs=2) as p2w, \
                 tc.tile_pool(name="psGm", bufs=1, space="PSUM") as psGmP, \
                 tc.tile_pool(name="psGx", bufs=2, space="PSUM") as psGxP, \
                 tc.tile_pool(name="psX", bufs=2, space="PSUM") as psXP, \
                 tc.tile_pool(name="psB", bufs=3, space="PSUM") as psBP:
                PH = [{} for _ in range(2)]
                for i in range(2):
                    ph = PH[i]
                    GTz = p2.tile([128, NCH * 129], f32, name="GTz")
                    nc.vector.memset(
                        GTz[:, :].rearrange("p (c z) -> p c z", z=129)[:, :, 0:1], 0.0)
                    for c in range(NCH):
                        nc.vector.tensor_tensor_scan(
                            out=GTz[:, c * 129 + 1:(c + 1) * 129],
                            data0=ones32[:, 0:128],
                            data1=gT[i][:, c * 128:(c + 1) * 128],
                            initial=0.0, op0=OP.mult, op1=OP.add)
                    GT3 = GTz[:, :].rearrange("p (c z) -> p c z", z=129)
                    gview = GT3[:, :, 1:129]
                    zview = GT3[:, :, 0:128]

                    eGT = p2.tile([128, ROWS], bf16, name="eGT")
                    nc.scalar.activation(
                        out=eGT[:, :].rearrange("p (c t) -> p c t", t=128),
                        in_=gview, func=AF.Exp)
                    eGlast = p2.tile([128, NCH], f32, name=f"eGl{i}")
                    nc.scalar.activation(
                        out=eGlast[:, :].rearrange("p (c o) -> p c o", o=1),
                        in_=GT3[:, :, 128:129], func=AF.Exp)
                    ph["eGlast"] = eGlast

                    kn = p2.tile([128, ROWS], bf16, name="kn")
                    sq = p2u.tile([128, ROWS], f32, name="sq")
                    nc.scalar.activation(out=sq, in_=kc[i], func=AF.Square)
                    rrow = p2t.tile([1, ROWS], f32, name="rrow")
                    for hf in range(2):
                        psq = psGxP.tile([128, 512], f32, name="psGx")
                        nc.tensor.matmul(psq[0:1, :], lhsT=ones32[:, 0:1],
                                         rhs=sq[:, hf * 512:(hf + 1) * 512],
                                         start=True, stop=True)
                        nc.scalar.activation(out=rrow[0:1, hf * 512:(hf + 1) * 512],
                                             in_=psq[0:1, :], func=AF.Sqrt,
                                             bias=cst[0:1, 28:29])
                    nc.vector.reciprocal(out=rrow, in_=rrow)
                    for hf in range(2):
                        psbc = psGxP.tile([128, 512], f32, name="psGx")
                        for cc in range(4):
                            c = hf * 4 + cc
                            nc.tensor.matmul(psbc[:, cc * 128:(cc + 1) * 128],
                                             lhsT=ones32[0:1, 0:128],
                                             rhs=rrow[0:1, c * 128:(c + 1) * 128],
                                             start=True, stop=True)
                        nc.vector.tensor_tensor(out=kn[:, hf * 512:(hf + 1) * 512],
                                                in0=kc[i][:, hf * 512:(hf + 1) * 512],
                                                in1=psbc, op=OP.mult)
                    sqq = p2u.tile([128, ROWS], f32, name="sq")
                    nc.scalar.activation(out=sqq, in_=qc[i], func=AF.Square)
                    psrq = psBP.tile([128, 128], f32, name="psB")
                    for c in range(NCH):
                        nc.tensor.matmul(psrq[:, c:c + 1],
                                         lhsT=sqq[:, c * 128:(c + 1) * 128],
                                         rhs=ones32[:, 0:1],
                                         start=True, stop=True)
                    rqn = p2.tile([128, NCH], f32, name=f"rqn{i}")
                    nc.scalar.activation(out=rqn, in_=psrq[:, 0:NCH], func=AF.Sqrt,
                                         scale=float(D), bias=cst[:, 29:30])
                    nc.vector.reciprocal(out=rqn, in_=rqn)
                    ph["rqn"] = rqn
                    qs = qc[i]

                    KeGT = p2.tile([128, ROWS], bf16, name=f"KeGT{i}")
                    nc.vector.tensor_tensor(out=KeGT, in0=kn, in1=eGT, op=OP.mult)
                    QeGsT = p2.tile([128, ROWS], bf16, name=f"QeG{i}")
                    nc.vector.tensor_tensor(out=QeGsT, in0=qs, in1=eGT, op=OP.mult)
                    qk = p2.tile([128, ROWS], bf16, name="qk")
                    nc.vector.tensor_tensor(out=qk, in0=qs, in1=kn, op=OP.mult)
                    ph["KeGT"] = KeGT
                    ph["QeGsT"] = QeGsT

                    etT = p2t.tile([128, ROWS], bf16, name="etT")
                    for c in range(NCH):
                        nc.scalar.activation(out=etT[:, c * 128:(c + 1) * 128],
                                             in_=GTz[:, c * 129 + 1:(c + 1) * 129],
                                             func=AF.Exp, scale=-1.0,
                                             bias=GTz[:, c * 129 + 128:c * 129 + 129])
                    KetT = p2t.tile([128, ROWS], bf16, name="KetT")
                    nc.vector.tensor_tensor(out=KetT, in0=kn, in1=etT, op=OP.mult)
                    KetN = p2.tile([128, ROWS], bf16, name=f"KetN{i}")
                    VN = p2.tile([128, ROWS], bf16, name=f"VN{i}")
                    ph["KetN"] = KetN
                    ph["VN"] = VN
                    for c in range(NCH):
                        nc.sync.dma_start_transpose(
                            out=KetN[:, c * 128:(c + 1) * 128],
                            in_=KetT[:, c * 128:(c + 1) * 128])
                        nc.sync.dma_start_transpose(
                            out=VN[:, c * 128:(c + 1) * 128],
                            in_=vc[i][:, c * 128:(c + 1) * 128])

                    lvl_ops = []
                    for li, bs in enumerate((128, 64, 32, 16, 8)):
                        nb = 128 // bs
                        cen_off = bs // 2 if bs > 8 else 0
                        g4 = gview.rearrange("p c (nb bs) -> p c nb bs", bs=bs)
                        z4 = zview.rearrange("p c (nb bs) -> p c nb bs", bs=bs)
                        cen = z4[:, :, :, cen_off:cen_off + 1].broadcast_to(
                            [128, NCH, nb, bs])
                        dlt = p2u.tile([128, ROWS], f32, name="dlt")
                        nc.vector.tensor_tensor(
                            out=dlt[:, :].rearrange("p (c nb bs) -> p c nb bs",
                                                    c=NCH, bs=bs),
                            in0=g4, in1=cen, op=OP.subtract)
                        el = p2u.tile([128, ROWS], bf16, name="el")
                        er = p2u.tile([128, ROWS], bf16, name="er")
                        KEL = p2.tile([128, ROWS], bf16, name=f"KEL{li}")
                        KER = p2.tile([128, ROWS], bf16, name=f"KER{li}")
                        QEL = p2.tile([128, ROWS], bf16, name=f"QEL{li}")

                        def hv(t_, lohi):
                            v = t_[:, :].rearrange("p (c nb b2) -> p c nb b2",
                                                   c=NCH, b2=bs)
                            return (v[:, :, :, bs // 2:bs] if lohi else
                                    v[:, :, :, 0:bs // 2])

                        if li <= 1:
                            nc.scalar.activation(out=hv(el, 1), in_=hv(dlt, 1),
                                                 func=AF.Exp)
                            nc.scalar.activation(out=hv(er, 0), in_=hv(dlt, 0),
                                                 func=AF.Exp, scale=-1.0)
                            nc.vector.tensor_tensor(out=hv(KEL, 1), in0=hv(kn, 1),
                                                    in1=hv(el, 1), op=OP.mult)
                            nc.vector.tensor_tensor(out=hv(KER, 0), in0=hv(kn, 0),
                                                    in1=hv(er, 0), op=OP.mult)
                            nc.vector.tensor_tensor(out=hv(QEL, 1), in0=hv(qs, 1),
                                                    in1=hv(el, 1), op=OP.mult)
                        elif bs > 8:
                            nc.vector.memset(hv(KEL, 0), 0.0)
                            nc.vector.memset(hv(QEL, 0), 0.0)
                            nc.scalar.activation(out=hv(el, 1), in_=hv(dlt, 1),
                                                 func=AF.Exp)
                            nc.scalar.activation(out=er, in_=dlt, func=AF.Exp,
                                                 scale=-1.0)
                            nc.vector.tensor_tensor(out=hv(KEL, 1), in0=hv(kn, 1),
                                                    in1=hv(el, 1), op=OP.mult)
                            nc.vector.tensor_tensor(out=KER, in0=kn, in1=er,
                                                    op=OP.mult)
                            nc.vector.tensor_tensor(out=hv(QEL, 1), in0=hv(qs, 1),
                                                    in1=hv(el, 1), op=OP.mult)
                        else:
                            nc.scalar.activation(out=el, in_=dlt, func=AF.Exp)
                            tmx = p2u.tile([128, ROWS], f32, name="tmx")
                            nc.vector.tensor_scalar(out=tmx, in0=dlt, scalar1=-85.0,
                                                    scalar2=None, op0=OP.max)
                            nc.scalar.activation(out=er, in_=tmx, func=AF.Exp,
                                                 scale=-1.0)
                            nc.vector.tensor_tensor(out=KEL, in0=kn, in1=el, op=OP.mult)
                            nc.vector.tensor_tensor(out=KER, in0=kn, in1=er, op=OP.mult)
                            nc.vector.tensor_tensor(out=QEL, in0=qs, in1=el, op=OP.mult)
                        lvl_ops.append((KEL, KER, QEL))

                    TmT = [p2.tile([128, 128], bf16, name=f"TmT{i}_{c}")
                           for c in range(NCH)]
                    AqT = [p2.tile([128, 128], bf16, name=f"AqT{i}_{c}")
                           for c in range(NCH)]
                    ph["TmT"] = TmT
                    ph["AqT"] = AqT
                    Nb = p2.tile([128, ROWS], bf16, name="Nb")
                    KEL0, KER0, QEL0 = lvl_ops[0]
                    KEL1, KER1, QEL1 = lvl_ops[1]
                    KEL2, KER2, QEL2 = lvl_ops[2]
                    KEL3, KER3, QEL3 = lvl_ops[3]
                    KEL8, KER8, QEL8 = lvl_ops[4]
                    for half in range(2):
                        for orient in range(2):
                            psMain = psGmP.tile([128, 512], f32, name="psGm")
                            nc.vector.memset(psMain, 0.0)
                            psLs = []
                            for lvl_idx, (KA, KB) in enumerate((
                                    (KER2, KEL2) if orient == 0 else (KER2, QEL2),
                                    (KER3, KEL3) if orient == 0 else (KER3, QEL3),
                                    (KER8, KEL8) if orient == 0 else (KER8, QEL8))):
                                psL = psGxP.tile([128, 512], f32, name="psGx")
                                for cc in range(4):
                                    c = half * 4 + cc
                                    sl = slice(c * 128, (c + 1) * 128)
                                    nc.tensor.matmul(
                                        psL[:, cc * 128:(cc + 1) * 128],
                                        lhsT=KA[:, sl], rhs=KB[:, sl],
                                        start=True, stop=True)
                                psLs.append(psL)
                            RL0 = KEL0 if orient == 0 else QEL0
                            RL1 = KEL1 if orient == 0 else QEL1
                            for cc in range(4):
                                c = half * 4 + cc
                                o0 = cc * 128
                                nc.tensor.matmul(
                                    psMain[0:64, o0 + 64:o0 + 128],
                                    lhsT=KER0[:, c * 128:c * 128 + 64],
                                    rhs=RL0[:, c * 128 + 64:c * 128 + 128],
                                    start=True, stop=True)
                                nc.tensor.matmul(
                                    psMain[0:32, o0 + 32:o0 + 64],
                                    lhsT=KER1[:, c * 128:c * 128 + 32],
                                    rhs=RL1[:, c * 128 + 32:c * 128 + 64],
                                    start=True, stop=True)
                                nc.tensor.matmul(
                                    psMain[64:96, o0 + 96:o0 + 128],
                                    lhsT=KER1[:, c * 128 + 64:c * 128 + 96],
                                    rhs=RL1[:, c * 128 + 96:c * 128 + 128],
                                    start=True, stop=True)
                            a0 = p2w.tile([128, 512], f32, name="a0")
                            nc.scalar.copy(out=a0, in_=psMain)
                            a1 = p2w.tile([128, 512], f32, name="a1")
                            nc.vector.select(out=a1, mask=masksb["m32Tx4"],
                                             on_true=psLs[0], on_false=a0)
                            a3 = p2w.tile([128, 512], f32, name="a1")
                            nc.vector.select(out=a3, mask=masksb["m16Tx4"],
                                             on_true=psLs[1], on_false=a1)
                            gsum = p2w.tile([128, 512], f32, name="gsum")
                            nc.vector.select(out=gsum, mask=masksb["m8Tx4"],
                                             on_true=psLs[2], on_false=a3)
                            for cc in range(4):
                                c = half * 4 + cc
                                sl = slice(c * 128, (c + 1) * 128)
                                gsl = slice(cc * 128, (cc + 1) * 128)
                                if orient == 0:
                                    nc.scalar.mul(
                                        out=Nb[:, c * 128:(c + 1) * 128],
                                        in_=gsum[:, gsl],
                                        mul=betaN[:, 2 * c + i:2 * c + i + 1])
                                else:
                                    psd = psBP.tile([128, 128], f32, name="psB")
                                    nc.tensor.matmul(psd[:, 0:1], lhsT=qk[:, sl],
                                                     rhs=onesbf[:, 0:1],
                                                     start=True, stop=True)
                                    tmpA = p2s.tile([128, 128], bf16, name="tmpA")
                                    nc.scalar.copy(out=tmpA, in_=gsum[:, gsl])
                                    nc.vector.scalar_tensor_tensor(
                                        out=AqT[c], in0=idbf, scalar=psd[:, 0:1],
                                        in1=tmpA, op0=OP.mult, op1=OP.add)

                    idx3 = idbf[:, :].unsqueeze(1).broadcast_to([128, 4, 128])
                    for hf in range(2):
                        xprev = [idbf] * 4
                        for j in range(JH):
                            psXb = psXP.tile([128, 512], f32, name="psX")
                            for cc in range(4):
                                c = hf * 4 + cc
                                nc.tensor.matmul(psXb[:, cc * 128:(cc + 1) * 128],
                                                 lhsT=Nb[:, c * 128:(c + 1) * 128],
                                                 rhs=xprev[cc], start=True, stop=True)
                            xnb = p2s.tile([128, 512], bf16, name="xh")
                            nc.vector.scalar_tensor_tensor(
                                out=xnb[:, :].rearrange("p (c t) -> p c t", t=128),
                                in0=idx3, scalar=1.0,
                                in1=psXb[:, :].rearrange("p (c t) -> p c t", t=128),
                                op0=OP.mult, op1=OP.subtract)
                            xprev = [xnb[:, cc * 128:(cc + 1) * 128] for cc in range(4)]
                        for cc in range(4):
                            c = hf * 4 + cc
                            nc.sync.dma_start_transpose(out=TmT[c][:, :], in_=xprev[cc])

                    ogg = p2t.tile([128, ROWS], f32, name=f"ogg{i}")
                    ph["ogg"] = ogg
                    Sf = p2.tile([128, 128], f32, name=f"Sf{i}")
                    Sb = p2.tile([128, 128], bf16, name=f"Sb{i}")
                    nc.vector.memset(Sf, 0.0)
                    nc.vector.memset(Sb, 0.0)
                    ph["Sf"] = Sf
                    ph["Sb"] = Sb
                # serial chain for this head
                    for c in range(NCH):
                        sl = slice(c * 128, (c + 1) * 128)
                        Sf, Sb = ph["Sf"], ph["Sb"]
                        psKS = psBP.tile([128, 128], f32, name="psB")
                        nc.tensor.matmul(psKS[:, :], lhsT=ph["KeGT"][:, sl], rhs=Sb,
                                         start=True, stop=True)
                        Rr = p2s.tile([128, 128], bf16, name="Rr")
                        nc.vector.tensor_tensor(out=Rr, in0=ph["VN"][:, sl], in1=psKS,
                                                op=OP.subtract)
                        psY = psBP.tile([128, 128], f32, name="psB")
                        nc.tensor.matmul(psY[:, :], lhsT=ph["TmT"][c], rhs=Rr,
                                         start=True, stop=True)
                        Dl = p2s.tile([128, 128], bf16, name="Dl")
                        nc.scalar.mul(out=Dl, in_=psY,
                                      mul=betaN[:, 2 * c + i:2 * c + i + 1])
                        psO = psBP.tile([128, 128], f32, name="psB")
                        nc.tensor.matmul(psO[:, :], lhsT=ph["QeGsT"][:, sl], rhs=Sb,
                                         start=True, stop=False)
                        nc.tensor.matmul(psO[:, :], lhsT=ph["AqT"][c], rhs=Dl,
                                         start=False, stop=True)
                        psS = psBP.tile([128, 128], f32, name="psB")
                        nc.tensor.matmul(psS[:, :], lhsT=ph["KetN"][:, sl], rhs=Dl,
                                         start=True, stop=True)
                        nc.vector.scalar_tensor_tensor(
                            out=Sf, in0=Sf, scalar=ph["eGlast"][:, c:c + 1],
                            in1=psS, op0=OP.mult, op1=OP.add)
                        nc.scalar.copy(out=Sb, in_=Sf)
                        nc.vector.scalar_tensor_tensor(
                            out=ph["ogg"][:, sl], in0=psO,
                            scalar=ph["rqn"][:, c:c + 1],
                            in1=sig[i][:, sl], op0=OP.mult, op1=OP.mult)

                # batched gated-RMS + transpose per head
                    ogg = ph["ogg"]
                    sqo = p2u.tile([128, ROWS], f32, name="sqo")
                    nc.scalar.activation(out=sqo, in_=ogg, func=AF.Square)
                    ssr = p2s.tile([128, NCH], f32, name="ssr")
                    nc.vector.tensor_reduce(
                        out=ssr, in_=sqo[:, :].rearrange("p (c t) -> p c t", t=128),
                        axis=mybir.AxisListType.X, op=OP.add)
                    rmsr = p2s.tile([128, NCH], f32, name="rmsr")
                    nc.scalar.activation(out=rmsr, in_=ssr, func=AF.Sqrt,
                                         scale=1.0 / D, bias=cst[:, 28:29])
                    nc.vector.reciprocal(out=rmsr, in_=rmsr)
                    ogn = p2u.tile([128, ROWS], bf16, name="ogn")
                    nc.vector.tensor_tensor(
                        out=ogn[:, :].rearrange("p (c t) -> p c t", t=128),
                        in0=ogg[:, :].rearrange("p (c t) -> p c t", t=128),
                        in1=rmsr[:, :].unsqueeze(2).broadcast_to([128, NCH, 128]),
                        op=OP.mult)
                    for c in range(NCH):
                        sl = slice(c * 128, (c + 1) * 128)
                        nc.sync.dma_start_transpose(out=ogT[i][:, sl], in_=ogn[:, sl])


            # ================= phase 3: A2A + o_proj =================
            if debug_ogt:
                with tc.tile_pool(name="dbg", bufs=1) as dbp:
                    for i in range(2):
                        tmp = dbp.tile([128, ROWS], f32, name="dbgcp")
                        nc.scalar.copy(out=tmp, in_=ogT[i])
                        nc.sync.dma_start(out=d_ogt_dbg[i * 128:(i + 1) * 128, :], in_=tmp)

            if with_collective:
                with tc.tile_pool(name="ph3", bufs=1) as p3, \
                     tc.tile_pool(name="ph3t", bufs=2) as p3t, \
                     tc.tile_pool(name="ps3", bufs=2, space="PSUM") as ps3:
                    # shard j (rows 256j..256j+256) = [h0 | h1] cols 128j..128j+128
                    for j in range(8):
                        for i in range(2):
                            nc.sync.dma_start(
                                out=d_cin[256 * j + 128 * i:256 * j + 128 * (i + 1), :],
                                in_=ogT[i][:, 128 * j:128 * (j + 1)])
                    nc.gpsimd.collective_compute(
                        "AllToAll", mybir.AluOpType.bypass,
                        ins=[d_cin[:, :]], outs=[d_cout[:, :]],
                        replica_groups=[list(range(8))])
                    # cout rows [0:1024) = batch-0 og (heads 0..7), [1024:2048) batch-1
                    oga = p3.tile([128, 16 * 128], bf16, name="oga")
                    for kt in range(16):
                        nc.sync.dma_start(out=oga[:, kt * 128:(kt + 1) * 128],
                                          in_=d_cout[kt * 128:(kt + 1) * 128, :])
                    for b2 in range(2):
                        for nn2 in range(H // 512):
                            pso = ps3.tile([128, 512], f32, name="pso")
                            for kt in range(8):
                                nc.tensor.matmul(
                                    pso[:, :],
                                    lhsT=oga[:, (b2 * 8 + kt) * 128:
                                             (b2 * 8 + kt + 1) * 128],
                                    rhs=wo[:, kt * H + nn2 * 512:
                                           kt * H + (nn2 + 1) * 512],
                                    start=(kt == 0), stop=(kt == 7))
                            osb = p3t.tile([128, 512], f32, name="osb")
                            nc.scalar.copy(out=osb, in_=pso)
                            nc.sync.dma_start(
                                out=d_out[b2 * 128:(b2 + 1) * 128,
                                          nn2 * 512:(nn2 + 1) * 512],
                                in_=osb)

            pwo_cm.__exit__(None, None, None)

    if split_waits:
        _split_excess_waits(nc, mybir)
    return nc


def _host_inputs(inputs):
    x = np.asarray(inputs['hidden_states'], np.float32)
    Wq, Wk, Wv = (np.asarray(inputs[k], np.float32) for k in ('Wq', 'Wk', 'Wv'))
    conv_q, conv_k, conv_v = (np.asarray(inputs[k], np.float32)
                              for k in ('conv_q', 'conv_k', 'conv_v'))
    A_log = np.asarray(inputs['A_log'], np.float32)
    dt_bias = np.asarray(inputs['dt_bias'], np.float32)
    Wfa, Wfb = np.asarray(inputs['Wfa'], np.float32), np.asarray(inputs['Wfb'], np.float32)
    Wb = np.asarray(inputs['Wb'], np.float32)
    Wga, Wgb = np.asarray(inputs['Wga'], np.float32), np.asarray(inputs['Wgb'], np.float32)
    nw = np.asarray(inputs['norm_weight'], np.float32)
    Wo = np.asarray(inputs['Wo'], np.float32)
    WoT_folded = np.ascontiguousarray((Wo * np.tile(nw, NH)[None, :]).T)

    in_maps = []
    for cidx in range(NCORES):
        b, hp = cidx // 4, cidx % 4
        heads = (2 * hp, 2 * hp + 1)
        xT = np.ascontiguousarray(x[b].T)
        Wsl = []
        for h in heads: Wsl.append(Wq[128 * h:128 * (h + 1)])
        for h in heads: Wsl.append(Wk[128 * h:128 * (h + 1)])
        for h in heads: Wsl.append(Wv[128 * h:128 * (h + 1)])
        Wsl.append(Wfa); Wsl.append(Wga)
        Wsl.append(Wb[list(heads)])
        WprojT = np.ascontiguousarray(np.concatenate(Wsl, axis=0).T)
        consts = np.zeros((128, 32), np.float32)
        for i, h in enumerate(heads):
            consts[:, 12 * i + 0:12 * i + 4] = conv_q[128 * h:128 * (h + 1)]
            consts[:, 12 * i + 4:12 * i + 8] = conv_k[128 * h:128 * (h + 1)]
            consts[:, 12 * i + 8:12 * i + 12] = conv_v[128 * h:128 * (h + 1)]
            consts[:, 24 + i] = dt_bias[128 * h:128 * (h + 1)]
            consts[:, 26 + i] = -np.exp(A_log[h])
        consts[:, 28] = EPS
        consts[:, 29] = D * EPS
        wfb2 = np.concatenate([Wfb[128 * h:128 * (h + 1)].T for h in heads], axis=1)
        wgb2 = np.concatenate([Wgb[128 * h:128 * (h + 1)].T for h in heads], axis=1)
        in_maps.append({
            "xt": xT.astype(BF16),
            "wp": WprojT.astype(BF16),
            "wfb": np.ascontiguousarray(wfb2).astype(BF16),
            "wgb": np.ascontiguousarray(wgb2).astype(BF16),
            "wo": WoT_folded.astype(BF16),
            "cst": consts,
        })
    return in_maps


def _estimate_exec_ns(nc):
    """Best-effort single-core HW time estimate from the instruction cost
    model (NTFF profiling is unavailable under the axon client)."""
    try:
        from concourse.timeline_sim import TimelineSim
        return int(TimelineSim(nc, trace=False).simulate())
    except Exception:
        return None


def kernel(**inputs):
    global LAST_EXEC_NS
    from concourse.bass_utils import run_bass_kernel_spmd

    if "nc" not in _CACHE:
        _CACHE["nc"] = build_nc()
    nc = _CACHE["nc"]
    in_maps = _host_inputs(inputs)
    res = run_bass_kernel_spmd(nc, in_maps, core_ids=list(range(NCORES)), trace=False)
    if res.exec_time_ns is not None:
        LAST_EXEC_NS = res.exec_time_ns
    else:
        LAST_EXEC_NS = _estimate_exec_ns(nc)
    out = np.zeros((B * T, H), np.float32)
    for cidx in range(NCORES):
        r = res.results[cidx]["out"]
        out[128 * cidx:128 * (cidx + 1)] = r[0:128]
        out[T + 128 * cidx:T + 128 * (cidx + 1)] = r[128:256]
    return out



# revision 5
# speedup vs baseline: 1.0727x; 1.0113x over previous
"""nn_KimiDecoderLayer on 8 TRN2 NeuronCores, fully on-device.

Sharding: TP4 over heads x DP2 over batch. Core c in 0..3: batch 0, heads
(2c, 2c+1); core c+4: batch 1, same heads. Projections, short conv, gated
delta-rule recurrence (chunked, exact hierarchical decay factorization
matching the reference), gated RMS norm run head-local; o_proj runs
row-parallel after an in-kernel AllToAll of the normed outputs.
"""
import numpy as np
import ml_dtypes

B, T, H = 2, 1024, 2048
NH, D = 8, 128
P = NH * D
K = 4
EPS = 1e-6
SCALE = D ** -0.5
C = 128
JH = 2
NCORES = 8
ROWS = 1024          # rows per core (one batch)
NCH = 8              # chunks per head

LAST_EXEC_NS = None
_CACHE = {}

BF16 = ml_dtypes.bfloat16


def _split_excess_waits(nc, mybir, max_waits=1):
    """walrus in this env rejects >1 sem-wait per instruction; hoist extras
    onto preceding same-engine NoOps (semantically identical)."""
    n = 0
    for f in nc.m.functions:
        for blk in f.blocks:
            new = []
            changed = False
            for inst in blk.instructions:
                si = inst.sync_info
                if si is not None and si.on_wait is not None and len(si.on_wait) > max_waits:
                    waits = list(si.on_wait)
                    groups = [waits[i:i + max_waits] for i in range(0, len(waits), max_waits)]
                    for gi, g in enumerate(groups[:-1]):
                        new.append(mybir.InstNoOp(
                            name=f"{inst.name}-ws{gi}", engine=inst.engine,
                            sync_info=mybir.SyncInfo(on_wait=g, on_update=[]),
                            bass_nofuse=True))
                    inst.sync_info = mybir.SyncInfo(on_wait=groups[-1],
                                                    on_update=list(si.on_update))
                    changed = True
                    n += 1
                new.append(inst)
            if changed:
                blk.instructions = new
    return n


def _masks():
    def rect(bs):
        m = np.zeros((C, C), np.float32)
        for b0 in range(0, C, bs):
            m[b0 + bs // 2:b0 + bs, b0:b0 + bs // 2] = 1
        return m
    m32, m16 = rect(32), rect(16)
    mask8 = np.zeros((C, C), np.float32)
    for b0 in range(0, C, 8):
        mask8[b0:b0 + 8, b0:b0 + 8] = 1
    m8 = np.tril(mask8, -1)
    return m32, m16, m8


def build_nc(with_collective=True, debug_ogt=False, split_waits=True):
    import concourse.bass as bass
    import concourse.tile as tile
    from concourse import mybir

    f32 = mybir.dt.float32
    bf16 = mybir.dt.bfloat16

    nc = bass.Bass(num_devices=NCORES)
    d_xt = nc.dram_tensor("xt", (H, ROWS), bf16, kind="ExternalInput")
    d_wp = nc.dram_tensor("wp", (H, 1026), bf16, kind="ExternalInput")
    d_wfb = nc.dram_tensor("wfb", (128, 256), bf16, kind="ExternalInput")
    d_wgb = nc.dram_tensor("wgb", (128, 256), bf16, kind="ExternalInput")
    d_wo = nc.dram_tensor("wo", (P, H), bf16, kind="ExternalInput")
    d_cst = nc.dram_tensor("cst", (128, 32), f32, kind="ExternalInput")
    d_out = nc.dram_tensor("out", (256, H), f32, kind="ExternalOutput")
    d_ogt_dbg = None
    if debug_ogt:
        d_ogt_dbg = nc.dram_tensor("ogt_dbg", (256, ROWS), f32, kind="ExternalOutput")
    if with_collective:
        d_cin = nc.dram_tensor("a2a_in", (2 * P, 128), bf16, kind="Internal")
        d_cout = nc.dram_tensor("a2a_out", (2 * P, 128), bf16, kind="Internal")

    m32_np, m16_np, m8_np = _masks()
    d_m = {}
    d_mnp = {"m32": m32_np, "m16": m16_np, "m8": m8_np,
             "m32T": np.ascontiguousarray(m32_np.T),
             "m16T": np.ascontiguousarray(m16_np.T),
             "m8T": np.ascontiguousarray(m8_np.T)}
    for nm, arr in d_mnp.items():
        d_m[nm] = nc.inline_tensor(arr.astype(np.float32), name=nm)
    d_m4 = {}
    for nm in ("m32T", "m16T", "m8T"):
        arr4 = np.tile(d_mnp[nm], (1, 4))
        d_m4[nm] = nc.inline_tensor(arr4.astype(np.uint8), name=nm + "x4")
    d_id_bf = nc.inline_tensor(np.eye(128, dtype=BF16), name="idbf")
    d_id_f32 = nc.inline_tensor(np.eye(128, dtype=np.float32), name="idf32")
    d_ones_f32 = nc.inline_tensor(np.ones((128, 128), np.float32), name="ones32")
    d_ones_bf = nc.inline_tensor(np.ones((128, 1), BF16), name="onesbf")

    AF = mybir.ActivationFunctionType
    OP = mybir.AluOpType

    with tile.TileContext(nc) as tc:
        with tc.tile_pool(name="wpool", bufs=1) as wp:
            masksb = {}
            u8 = mybir.dt.uint8
            for nm in ("m32T", "m16T", "m8T"):
                mt = wp.tile([128, 512], u8, name=nm + "x4")
                nc.sync.dma_start(out=mt, in_=d_m4[nm][:, :])
                masksb[nm + "x4"] = mt
            idbf = wp.tile([128, 128], bf16, name="idbf")
            nc.sync.dma_start(out=idbf, in_=d_id_bf[:, :])
            idf32 = wp.tile([128, 128], f32, name="idf32")
            nc.sync.dma_start(out=idf32, in_=d_id_f32[:, :])
            ones32 = wp.tile([128, 128], f32, name="ones32")
            nc.sync.dma_start(out=ones32, in_=d_ones_f32[:, :])
            onesbf = wp.tile([128, 1], bf16, name="onesbf")
            nc.sync.dma_start(out=onesbf, in_=d_ones_bf[:, :])
            cst = wp.tile([128, 32], f32, name="cst")
            nc.sync.dma_start(out=cst, in_=d_cst[:, :])

            qc = [wp.tile([128, ROWS], bf16, name=f"qc{i}") for i in range(2)]
            kc = [wp.tile([128, ROWS], bf16, name=f"kc{i}") for i in range(2)]
            vc = [wp.tile([128, ROWS], bf16, name=f"vc{i}") for i in range(2)]
            gT = [wp.tile([128, ROWS], f32, name=f"gT{i}") for i in range(2)]
            sig = [wp.tile([128, ROWS], bf16, name=f"sig{i}") for i in range(2)]
            betaT = wp.tile([2, ROWS], f32, name="betaT")
            betaN = wp.tile([128, 16], f32, name="betaN")
            ogT = [wp.tile([128, ROWS], bf16, name=f"ogT{i}") for i in range(2)]

            # ================= phase 1: projections =================
            with tc.tile_pool(name="ph1", bufs=1) as p1, \
                 tc.tile_pool(name="ph1t", bufs=2) as p1t, \
                 tc.tile_pool(name="ps1", bufs=2, space="PSUM") as ps1:
                xt = p1.tile([128, 16 * ROWS], bf16, name="xt")
                for kt in range(16):
                    nc.sync.dma_start(out=xt[:, kt * ROWS:(kt + 1) * ROWS],
                                      in_=d_xt[kt * 128:(kt + 1) * 128, :])
                wpj = p1.tile([128, 16 * 1026], bf16, name="wpj")
                for kt in range(16):
                    nc.sync.dma_start(out=wpj[:, kt * 1026:(kt + 1) * 1026],
                                      in_=d_wp[kt * 128:(kt + 1) * 128, :])
                wfb = p1.tile([128, 256], bf16, name="wfb")
                nc.sync.dma_start(out=wfb, in_=d_wfb[:, :])
                wgb = p1.tile([128, 256], bf16, name="wgb")
                nc.sync.dma_start(out=wgb, in_=d_wgb[:, :])

                qr = [p1.tile([128, ROWS + 3], bf16, name=f"qr{i}") for i in range(2)]
                kr = [p1.tile([128, ROWS + 3], bf16, name=f"kr{i}") for i in range(2)]
                vr = [p1.tile([128, ROWS + 3], bf16, name=f"vr{i}") for i in range(2)]
                fafT = p1.tile([128, ROWS], bf16, name="fafT")
                fagT = p1.tile([128, ROWS], bf16, name="fagT")
                raws = [qr[0], qr[1], kr[0], kr[1], vr[0], vr[1]]
                for t_ in raws:
                    nc.vector.memset(t_[:, 0:3], 0.0)

                for mc in (6, 7, 8, 0, 2, 4, 1, 3, 5):
                    m0 = mc * 128
                    msz = 2 if mc == 8 else 128
                    for nn in range(ROWS // 512):
                        ps = ps1.tile([128, 512], f32, name="pj")
                        for kt in range(16):
                            nc.tensor.matmul(
                                ps[0:msz, :],
                                lhsT=wpj[:, kt * 1026 + m0: kt * 1026 + m0 + msz],
                                rhs=xt[:, kt * ROWS + nn * 512: kt * ROWS + (nn + 1) * 512],
                                start=(kt == 0), stop=(kt == 15))
                        if mc < 6:
                            nc.scalar.copy(out=raws[mc][:, 3 + nn * 512: 3 + (nn + 1) * 512],
                                           in_=ps[:, :])
                        elif mc == 6:
                            nc.scalar.copy(out=fafT[:, nn * 512:(nn + 1) * 512], in_=ps[:, :])
                        elif mc == 7:
                            nc.scalar.copy(out=fagT[:, nn * 512:(nn + 1) * 512], in_=ps[:, :])
                        else:
                            nc.scalar.activation(out=betaT[:, nn * 512:(nn + 1) * 512],
                                                 in_=ps[0:2, :], func=AF.Sigmoid)

                for c in range(NCH):
                    psb = ps1.tile([128, 512], f32, name="pj")
                    nc.tensor.matmul(psb[0:128, 0:2],
                                     lhsT=betaT[0:2, c * 128:(c + 1) * 128],
                                     rhs=idf32[0:2, 0:2], is_transpose=True)
                    nc.scalar.copy(out=betaN[:, 2 * c:2 * c + 2], in_=psb[:, 0:2])

                for i in range(2):
                    for raw, out_t, cbase in ((qr[i], qc[i], 12 * i),
                                              (kr[i], kc[i], 12 * i + 4),
                                              (vr[i], vc[i], 12 * i + 8)):
                        acc = p1t.tile([128, ROWS], f32, name="cacc")
                        nc.scalar.mul(out=acc, in_=raw[:, 0:ROWS],
                                      mul=cst[:, cbase:cbase + 1])
                        for j in range(1, K):
                            acc2 = p1t.tile([128, ROWS], f32, name="cacc")
                            nc.vector.scalar_tensor_tensor(
                                out=acc2, in0=raw[:, j:j + ROWS],
                                scalar=cst[:, cbase + j:cbase + j + 1],
                                in1=acc, op0=OP.mult, op1=OP.add)
                            acc = acc2
                        sg = p1t.tile([128, ROWS], bf16, name="csig")
                        nc.scalar.activation(out=sg, in_=acc, func=AF.Sigmoid)
                        nc.vector.tensor_tensor(out=out_t, in0=acc, in1=sg, op=OP.mult)

                for i in range(2):
                    for nn in range(ROWS // 512):
                        ps = ps1.tile([128, 512], f32, name="pj")
                        nc.tensor.matmul(ps[:, :], lhsT=wfb[:, 128 * i:128 * (i + 1)],
                                         rhs=fafT[:, nn * 512:(nn + 1) * 512],
                                         start=True, stop=True)
                        tmp2 = p1t.tile([128, 512], f32, name="gtmp2")
                        nc.scalar.activation(out=tmp2, in_=ps[:, :], func=AF.Exp,
                                             bias=cst[:, 24 + i:25 + i])
                        tmp3 = p1t.tile([128, 512], f32, name="gtmp3")
                        nc.scalar.activation(out=tmp3, in_=tmp2, func=AF.Ln, bias=1.0)
                        nc.vector.tensor_scalar(out=gT[i][:, nn * 512:(nn + 1) * 512],
                                                in0=tmp3, scalar1=cst[:, 26 + i:27 + i],
                                                scalar2=None, op0=OP.mult)

                for i in range(2):
                    for hf in range(2):
                        ps = ps1.tile([128, 512], f32, name="pj")
                        for cc in range(4):
                            c = hf * 4 + cc
                            nc.tensor.matmul(ps[:, cc * 128:(cc + 1) * 128],
                                             lhsT=fagT[:, c * 128:(c + 1) * 128],
                                             rhs=wgb[:, 128 * i:128 * (i + 1)],
                                             start=True, stop=True)
                        nc.scalar.activation(out=sig[i][:, hf * 512:(hf + 1) * 512],
                                             in_=ps[:, :], func=AF.Sigmoid)

            # ================= phase 2: recurrence =================
            pwo_cm = tc.tile_pool(name="phwo", bufs=1)
            pwo = pwo_cm.__enter__()
            wo = None
            if with_collective:
                wo = pwo.tile([128, 8 * H], bf16, name="wo")
                for kt in range(8):
                    nc.sync.dma_start(out=wo[:, kt * H:(kt + 1) * H],
                                      in_=d_wo[kt * 128:(kt + 1) * 128, :])
            with tc.tile_pool(name="ph2u", bufs=1) as p2u, \
                 tc.tile_pool(name="ph2", bufs=1) as p2, \
                 tc.tile_pool(name="ph2t", bufs=2) as p2t, \
                 tc.tile_pool(name="ph2s", bufs=3) as p2s, \
                 tc.tile_pool(name="ph2w", bufs=2) as p2w, \
                 tc.tile_pool(name="psGm", bufs=1, space="PSUM") as psGmP, \
                 tc.tile_pool(name="psGx", bufs=2, space="PSUM") as psGxP, \
                 tc.tile_pool(name="psX", bufs=2, space="PSUM") as psXP, \
                 tc.tile_pool(name="psB", bufs=3, space="PSUM") as psBP:
                PH = [{} for _ in range(2)]
                for i in range(2):
                    ph = PH[i]
                    GTz = p2.tile([128, NCH * 129], f32, name="GTz")
                    nc.vector.memset(
                        GTz[:, :].rearrange("p (c z) -> p c z", z=129)[:, :, 0:1], 0.0)
                    for c in range(NCH):
                        nc.vector.tensor_tensor_scan(
                            out=GTz[:, c * 129 + 1:(c + 1) * 129],
                            data0=ones32[:, 0:128],
                            data1=gT[i][:, c * 128:(c + 1) * 128],
                            initial=0.0, op0=OP.mult, op1=OP.add)
                    GT3 = GTz[:, :].rearrange("p (c z) -> p c z", z=129)
                    gview = GT3[:, :, 1:129]
                    zview = GT3[:, :, 0:128]

                    eGT = p2.tile([128, ROWS], bf16, name="eGT")
                    nc.scalar.activation(
                        out=eGT[:, :].rearrange("p (c t) -> p c t", t=128),
                        in_=gview, func=AF.Exp)
                    eGlast = p2.tile([128, NCH], f32, name=f"eGl{i}")
                    nc.scalar.activation(
                        out=eGlast[:, :].rearrange("p (c o) -> p c o", o=1),
                        in_=GT3[:, :, 128:129], func=AF.Exp)
                    ph["eGlast"] = eGlast

                    kn = p2.tile([128, ROWS], bf16, name="kn")
                    sq = p2u.tile([128, ROWS], f32, name="sq")
                    nc.scalar.activation(out=sq, in_=kc[i], func=AF.Square)
                    rrow = p2t.tile([1, ROWS], f32, name="rrow")
                    for hf in range(2):
                        psq = psGxP.tile([128, 512], f32, name="psGx")
                        nc.tensor.matmul(psq[0:1, :], lhsT=ones32[:, 0:1],
                                         rhs=sq[:, hf * 512:(hf + 1) * 512],
                                         start=True, stop=True)
                        nc.scalar.activation(out=rrow[0:1, hf * 512:(hf + 1) * 512],
                                             in_=psq[0:1, :], func=AF.Sqrt,
                                             bias=cst[0:1, 28:29])
                    nc.vector.reciprocal(out=rrow, in_=rrow)
                    for hf in range(2):
                        psbc = psGxP.tile([128, 512], f32, name="psGx")
                        for cc in range(4):
                            c = hf * 4 + cc
                            nc.tensor.matmul(psbc[:, cc * 128:(cc + 1) * 128],
                                             lhsT=ones32[0:1, 0:128],
                                             rhs=rrow[0:1, c * 128:(c + 1) * 128],
                                             start=True, stop=True)
                        nc.vector.tensor_tensor(out=kn[:, hf * 512:(hf + 1) * 512],
                                                in0=kc[i][:, hf * 512:(hf + 1) * 512],
                                                in1=psbc, op=OP.mult)
                    sqq = p2u.tile([128, ROWS], f32, name="sq")
                    nc.scalar.activation(out=sqq, in_=qc[i], func=AF.Square)
                    psrq = psBP.tile([128, 128], f32, name="psB")
                    for c in range(NCH):
                        nc.tensor.matmul(psrq[:, c:c + 1],
                                         lhsT=sqq[:, c * 128:(c + 1) * 128],
                                         rhs=ones32[:, 0:1],
                                         start=True, stop=True)
                    rqn = p2.tile([128, NCH], f32, name=f"rqn{i}")
                    nc.scalar.activation(out=rqn, in_=psrq[:, 0:NCH], func=AF.Sqrt,
                                         scale=float(D), bias=cst[:, 29:30])
                    nc.vector.reciprocal(out=rqn, in_=rqn)
                    ph["rqn"] = rqn
                    qs = qc[i]

                    KeGT = p2.tile([128, ROWS], bf16, name=f"KeGT{i}")
                    nc.vector.tensor_tensor(out=KeGT, in0=kn, in1=eGT, op=OP.mult)
                    QeGsT = p2.tile([128, ROWS], bf16, name=f"QeG{i}")
                    nc.vector.tensor_tensor(out=QeGsT, in0=qs, in1=eGT, op=OP.mult)
                    qk = p2.tile([128, ROWS], bf16, name="qk")
                    nc.vector.tensor_tensor(out=qk, in0=qs, in1=kn, op=OP.mult)
                    ph["KeGT"] = KeGT
                    ph["QeGsT"] = QeGsT

                    etT = p2t.tile([128, ROWS], bf16, name="etT")
                    for c in range(NCH):
                        nc.scalar.activation(out=etT[:, c * 128:(c + 1) * 128],
                                             in_=GTz[:, c * 129 + 1:(c + 1) * 129],
                                             func=AF.Exp, scale=-1.0,
                                             bias=GTz[:, c * 129 + 128:c * 129 + 129])
                    KetT = p2t.tile([128, ROWS], bf16, name="KetT")
                    nc.vector.tensor_tensor(out=KetT, in0=kn, in1=etT, op=OP.mult)
                    KetN = p2.tile([128, ROWS], bf16, name=f"KetN{i}")
                    VN = p2.tile([128, ROWS], bf16, name=f"VN{i}")
                    ph["KetN"] = KetN
                    ph["VN"] = VN
                    for c in range(NCH):
                        nc.sync.dma_start_transpose(
                            out=KetN[:, c * 128:(c + 1) * 128],
                            in_=KetT[:, c * 128:(c + 1) * 128])
                        nc.sync.dma_start_transpose(
                            out=VN[:, c * 128:(c + 1) * 128],
                            in_=vc[i][:, c * 128:(c + 1) * 128])

                    lvl_ops = []
                    for li, bs in enumerate((128, 64, 32, 16, 8)):
                        nb = 128 // bs
                        cen_off = bs // 2 if bs > 8 else 0
                        g4 = gview.rearrange("p c (nb bs) -> p c nb bs", bs=bs)
                        z4 = zview.rearrange("p c (nb bs) -> p c nb bs", bs=bs)
                        cen = z4[:, :, :, cen_off:cen_off + 1].broadcast_to(
                            [128, NCH, nb, bs])
                        dlt = p2u.tile([128, ROWS], f32, name="dlt")
                        nc.vector.tensor_tensor(
                            out=dlt[:, :].rearrange("p (c nb bs) -> p c nb bs",
                                                    c=NCH, bs=bs),
                            in0=g4, in1=cen, op=OP.subtract)
                        el = p2u.tile([128, ROWS], bf16, name="el")
                        er = p2u.tile([128, ROWS], bf16, name="er")
                        KEL = p2.tile([128, ROWS], bf16, name=f"KEL{li}")
                        KER = p2.tile([128, ROWS], bf16, name=f"KER{li}")
                        QEL = p2.tile([128, ROWS], bf16, name=f"QEL{li}")

                        def hv(t_, lohi):
                            v = t_[:, :].rearrange("p (c nb b2) -> p c nb b2",
                                                   c=NCH, b2=bs)
                            return (v[:, :, :, bs // 2:bs] if lohi else
                                    v[:, :, :, 0:bs // 2])

                        if li <= 1:
                            nc.scalar.activation(out=hv(el, 1), in_=hv(dlt, 1),
                                                 func=AF.Exp)
                            nc.scalar.activation(out=hv(er, 0), in_=hv(dlt, 0),
                                                 func=AF.Exp, scale=-1.0)
                            nc.vector.tensor_tensor(out=hv(KEL, 1), in0=hv(kn, 1),
                                                    in1=hv(el, 1), op=OP.mult)
                            nc.vector.tensor_tensor(out=hv(KER, 0), in0=hv(kn, 0),
                                                    in1=hv(er, 0), op=OP.mult)
                            nc.vector.tensor_tensor(out=hv(QEL, 1), in0=hv(qs, 1),
                                                    in1=hv(el, 1), op=OP.mult)
                        elif bs > 8:
                            nc.vector.memset(hv(KEL, 0), 0.0)
                            nc.vector.memset(hv(QEL, 0), 0.0)
                            nc.scalar.activation(out=hv(el, 1), in_=hv(dlt, 1),
                                                 func=AF.Exp)
                            nc.scalar.activation(out=er, in_=dlt, func=AF.Exp,
                                                 scale=-1.0)
                            nc.vector.tensor_tensor(out=hv(KEL, 1), in0=hv(kn, 1),
                                                    in1=hv(el, 1), op=OP.mult)
                            nc.vector.tensor_tensor(out=KER, in0=kn, in1=er,
                                                    op=OP.mult)
                            nc.vector.tensor_tensor(out=hv(QEL, 1), in0=hv(qs, 1),
                                                    in1=hv(el, 1), op=OP.mult)
                        else:
                            nc.scalar.activation(out=el, in_=dlt, func=AF.Exp)
                            tmx = p2u.tile([128, ROWS], f32, name="tmx")
                            nc.vector.tensor_scalar(out=tmx, in0=dlt, scalar1=-85.0,
                                                    scalar2=None, op0=OP.max)
                            nc.scalar.activation(out=er, in_=tmx, func=AF.Exp,
                                                 scale=-1.0)
                            nc.vector.tensor_tensor(out=KEL, in0=kn, in1=el, op=OP.mult)
                            nc.vector.tensor_tensor(out=KER, in0=kn, in1=er, op=OP.mult)
                            nc.vector.tensor_tensor(out=QEL, in0=qs, in1=el, op=OP.mult)
                        lvl_ops.append((KEL, KER, QEL))

                    TmT = [p2.tile([128, 128], bf16, name=f"TmT{i}_{c}")
                           for c in range(NCH)]
                    AqT = [p2.tile([128, 128], bf16, name=f"AqT{i}_{c}")
                           for c in range(NCH)]
                    ph["TmT"] = TmT
                    ph["AqT"] = AqT
                    Nb = p2.tile([128, ROWS], bf16, name="Nb")
                    KEL0, KER0, QEL0 = lvl_ops[0]
                    KEL1, KER1, QEL1 = lvl_ops[1]
                    KEL2, KER2, QEL2 = lvl_ops[2]
                    KEL3, KER3, QEL3 = lvl_ops[3]
                    KEL8, KER8, QEL8 = lvl_ops[4]
                    for half in range(2):
                        for orient in range(2):
                            psMain = psGmP.tile([128, 512], f32, name="psGm")
                            nc.vector.memset(psMain, 0.0)
                            psLs = []
                            for lvl_idx, (KA, KB) in enumerate((
                                    (KER2, KEL2) if orient == 0 else (KER2, QEL2),
                                    (KER3, KEL3) if orient == 0 else (KER3, QEL3),
                                    (KER8, KEL8) if orient == 0 else (KER8, QEL8))):
                                psL = psGxP.tile([128, 512], f32, name="psGx")
                                for cc in range(4):
                                    c = half * 4 + cc
                                    sl = slice(c * 128, (c + 1) * 128)
                                    nc.tensor.matmul(
                                        psL[:, cc * 128:(cc + 1) * 128],
                                        lhsT=KA[:, sl], rhs=KB[:, sl],
                                        start=True, stop=True)
                                psLs.append(psL)
                            RL0 = KEL0 if orient == 0 else QEL0
                            RL1 = KEL1 if orient == 0 else QEL1
                            for cc in range(4):
                                c = half * 4 + cc
                                o0 = cc * 128
                                nc.tensor.matmul(
                                    psMain[0:64, o0 + 64:o0 + 128],
                                    lhsT=KER0[:, c * 128:c * 128 + 64],
                                    rhs=RL0[:, c * 128 + 64:c * 128 + 128],
                                    start=True, stop=True)
                                nc.tensor.matmul(
                                    psMain[0:32, o0 + 32:o0 + 64],
                                    lhsT=KER1[:, c * 128:c * 128 + 32],
                                    rhs=RL1[:, c * 128 + 32:c * 128 + 64],
                                    start=True, stop=True)
                                nc.tensor.matmul(
                                    psMain[64:96, o0 + 96:o0 + 128],
                                    lhsT=KER1[:, c * 128 + 64:c * 128 + 96],
                                    rhs=RL1[:, c * 128 + 96:c * 128 + 128],
                                    start=True, stop=True)
                            a0 = p2w.tile([128, 512], f32, name="a0")
                            nc.scalar.copy(out=a0, in_=psMain)
                            a1 = p2w.tile([128, 512], f32, name="a1")
                            nc.vector.select(out=a1, mask=masksb["m32Tx4"],
                                             on_true=psLs[0], on_false=a0)
                            a3 = p2w.tile([128, 512], f32, name="a1")
                            nc.vector.select(out=a3, mask=masksb["m16Tx4"],
                                             on_true=psLs[1], on_false=a1)
                            gsum = p2w.tile([128, 512], f32, name="gsum")
                            nc.vector.select(out=gsum, mask=masksb["m8Tx4"],
                                             on_true=psLs[2], on_false=a3)
                            for cc in range(4):
                                c = half * 4 + cc
                                sl = slice(c * 128, (c + 1) * 128)
                                gsl = slice(cc * 128, (cc + 1) * 128)
                                if orient == 0:
                                    nc.scalar.mul(
                                        out=Nb[:, c * 128:(c + 1) * 128],
                                        in_=gsum[:, gsl],
                                        mul=betaN[:, 2 * c + i:2 * c + i + 1])
                                else:
                                    psd = psBP.tile([128, 128], f32, name="psB")
                                    nc.tensor.matmul(psd[:, 0:1], lhsT=qk[:, sl],
                                                     rhs=onesbf[:, 0:1],
                                                     start=True, stop=True)
                                    tmpA = p2s.tile([128, 128], bf16, name="tmpA")
                                    nc.scalar.copy(out=tmpA, in_=gsum[:, gsl])
                                    nc.vector.scalar_tensor_tensor(
                                        out=AqT[c], in0=idbf, scalar=psd[:, 0:1],
                                        in1=tmpA, op0=OP.mult, op1=OP.add)

                    idx3 = idbf[:, :].unsqueeze(1).broadcast_to([128, 4, 128])
                    for hf in range(2):
                        xprev = [idbf] * 4
                        for j in range(JH):
                            psXb = psXP.tile([128, 512], f32, name="psX")
                            for cc in range(4):
                                c = hf * 4 + cc
                                nc.tensor.matmul(psXb[:, cc * 128:(cc + 1) * 128],
                                                 lhsT=Nb[:, c * 128:(c + 1) * 128],
                                                 rhs=xprev[cc], start=True, stop=True)
                            xnb = p2s.tile([128, 512], bf16, name="xh")
                            nc.vector.scalar_tensor_tensor(
                                out=xnb[:, :].rearrange("p (c t) -> p c t", t=128),
                                in0=idx3, scalar=1.0,
                                in1=psXb[:, :].rearrange("p (c t) -> p c t", t=128),
                                op0=OP.mult, op1=OP.subtract)
                            xprev = [xnb[:, cc * 128:(cc + 1) * 128] for cc in range(4)]
                        for cc in range(4):
                            c = hf * 4 + cc
                            nc.sync.dma_start_transpose(out=TmT[c][:, :], in_=xprev[cc])

                    ogg = p2t.tile([128, ROWS], f32, name=f"ogg{i}")
                    ph["ogg"] = ogg
                    Sf = p2.tile([128, 128], f32, name=f"Sf{i}")
                    Sb = p2.tile([128, 128], bf16, name=f"Sb{i}")
                    nc.vector.memset(Sf, 0.0)
                    nc.vector.memset(Sb, 0.0)
                    ph["Sf"] = Sf
                    ph["Sb"] = Sb
                # serial chain for this head
                    for c in range(NCH):
                        sl = slice(c * 128, (c + 1) * 128)
                        Sf, Sb = ph["Sf"], ph["Sb"]
                        psKS = psBP.tile([128, 128], f32, name="psB")
                        nc.tensor.matmul(psKS[:, :], lhsT=ph["KeGT"][:, sl], rhs=Sb,
                                         start=True, stop=True)
                        Rr = p2s.tile([128, 128], bf16, name="Rr")
                        nc.vector.tensor_tensor(out=Rr, in0=ph["VN"][:, sl], in1=psKS,
                                                op=OP.subtract)
                        psY = psBP.tile([128, 128], f32, name="psB")
                        nc.tensor.matmul(psY[:, :], lhsT=ph["TmT"][c], rhs=Rr,
                                         start=True, stop=True)
                        Dl = p2s.tile([128, 128], bf16, name="Dl")
                        nc.scalar.mul(out=Dl, in_=psY,
                                      mul=betaN[:, 2 * c + i:2 * c + i + 1])
                        psO = psBP.tile([128, 128], f32, name="psB")
                        nc.tensor.matmul(psO[:, :], lhsT=ph["QeGsT"][:, sl], rhs=Sb,
                                         start=True, stop=False)
                        nc.tensor.matmul(psO[:, :], lhsT=ph["AqT"][c], rhs=Dl,
                                         start=False, stop=True)
                        psS = psBP.tile([128, 128], f32, name="psB")
                        nc.tensor.matmul(psS[:, :], lhsT=ph["KetN"][:, sl], rhs=Dl,
                                         start=True, stop=True)
                        nc.vector.scalar_tensor_tensor(
                            out=Sf, in0=Sf, scalar=ph["eGlast"][:, c:c + 1],
                            in1=psS, op0=OP.mult, op1=OP.add)
                        nc.scalar.copy(out=Sb, in_=Sf)
                        nc.vector.scalar_tensor_tensor(
                            out=ph["ogg"][:, sl], in0=psO,
                            scalar=ph["rqn"][:, c:c + 1],
                            in1=sig[i][:, sl], op0=OP.mult, op1=OP.mult)

                # batched gated-RMS + transpose per head
                    ogg = ph["ogg"]
                    sqo = p2u.tile([128, ROWS], f32, name="sqo")
                    nc.scalar.activation(out=sqo, in_=ogg, func=AF.Square)
                    ssr = p2s.tile([128, NCH], f32, name="ssr")
                    nc.vector.tensor_reduce(
                        out=ssr, in_=sqo[:, :].rearrange("p (c t) -> p c t", t=128),
                        axis=mybir.AxisListType.X, op=OP.add)
                    rmsr = p2s.tile([128, NCH], f32, name="rmsr")
                    nc.scalar.activation(out=rmsr, in_=ssr, func=AF.Sqrt,
                                         scale=1.0 / D, bias=cst[:, 28:29])
                    nc.vector.reciprocal(out=rmsr, in_=rmsr)
                    ogn = p2u.tile([128, ROWS], bf16, name="ogn")
                    nc.vector.tensor_tensor(
                        out=ogn[:, :].rearrange("p (c t) -> p c t", t=128),
                        in0=ogg[:, :].rearrange("p (c t) -> p c t", t=128),
                        in1=rmsr[:, :].unsqueeze(2).broadcast_to([128, NCH, 128]),
                        op=OP.mult)
                    for c in range(NCH):
                        sl = slice(c * 128, (c + 1) * 128)
                        nc.sync.dma_start_transpose(out=ogT[i][:, sl], in_=ogn[:, sl])


            # ================= phase 3: A2A + o_proj =================
            if debug_ogt:
                with tc.tile_pool(name="dbg", bufs=1) as dbp:
                    for i in range(2):
                        tmp = dbp.tile([128, ROWS], f32, name="dbgcp")
                        nc.scalar.copy(out=tmp, in_=ogT[i])
                        nc.sync.dma_start(out=d_ogt_dbg[i * 128:(i + 1) * 128, :], in_=tmp)

            if with_collective:
                with tc.tile_pool(name="ph3", bufs=1) as p3, \
                     tc.tile_pool(name="ph3t", bufs=2) as p3t, \
                     tc.tile_pool(name="ps3", bufs=2, space="PSUM") as ps3:
                    # shard j (rows 256j..256j+256) = [h0 | h1] cols 128j..128j+128
                    for j in range(8):
                        for i in range(2):
                            nc.sync.dma_start(
                                out=d_cin[256 * j + 128 * i:256 * j + 128 * (i + 1), :],
                                in_=ogT[i][:, 128 * j:128 * (j + 1)])
                    nc.gpsimd.collective_compute(
                        "AllToAll", mybir.AluOpType.bypass,
                        ins=[d_cin[:, :]], outs=[d_cout[:, :]],
                        replica_groups=[list(range(8))])
                    # cout rows [0:1024) = batch-0 og (heads 0..7), [1024:2048) batch-1
                    oga = p3.tile([128, 16 * 128], bf16, name="oga")
                    for kt in range(16):
                        nc.sync.dma_start(out=oga[:, kt * 128:(kt + 1) * 128],
                                          in_=d_cout[kt * 128:(kt + 1) * 128, :])
                    for b2 in range(2):
                        for nn2 in range(H // 512):
                            pso = ps3.tile([128, 512], f32, name="pso")
                            for kt in range(8):
                                nc.tensor.matmul(
                                    pso[:, :],
                                    lhsT=oga[:, (b2 * 8 + kt) * 128:
                                             (b2 * 8 + kt + 1) * 128],
                                    rhs=wo[:, kt * H + nn2 * 512:
                                           kt * H + (nn2 + 1) * 512],
                                    start=(kt == 0), stop=(kt == 7))
                            osb = p3t.tile([128, 512], f32, name="osb")
                            nc.scalar.copy(out=osb, in_=pso)
                            nc.sync.dma_start(
                                out=d_out[b2 * 128:(b2 + 1) * 128,
                                          nn2 * 512:(nn2 + 1) * 512],
                                in_=osb)

            pwo_cm.__exit__(None, None, None)

    if split_waits:
        _split_excess_waits(nc, mybir)
    return nc


def _host_inputs(inputs):
    x = np.asarray(inputs['hidden_states'], np.float32)
    Wq, Wk, Wv = (np.asarray(inputs[k], np.float32) for k in ('Wq', 'Wk', 'Wv'))
    conv_q, conv_k, conv_v = (np.asarray(inputs[k], np.float32)
                              for k in ('conv_q', 'conv_k', 'conv_v'))
    A_log = np.asarray(inputs['A_log'], np.float32)
    dt_bias = np.asarray(inputs['dt_bias'], np.float32)
    Wfa, Wfb = np.asarray(inputs['Wfa'], np.float32), np.asarray(inputs['Wfb'], np.float32)
    Wb = np.asarray(inputs['Wb'], np.float32)
    Wga, Wgb = np.asarray(inputs['Wga'], np.float32), np.asarray(inputs['Wgb'], np.float32)
    nw = np.asarray(inputs['norm_weight'], np.float32)
    Wo = np.asarray(inputs['Wo'], np.float32)
    WoT_folded = np.ascontiguousarray((Wo * np.tile(nw, NH)[None, :]).T)

    in_maps = []
    for cidx in range(NCORES):
        b, hp = cidx // 4, cidx % 4
        heads = (2 * hp, 2 * hp + 1)
        xT = np.ascontiguousarray(x[b].T)
        Wsl = []
        for h in heads: Wsl.append(Wq[128 * h:128 * (h + 1)])
        for h in heads: Wsl.append(Wk[128 * h:128 * (h + 1)])
        for h in heads: Wsl.append(Wv[128 * h:128 * (h + 1)])
        Wsl.append(Wfa); Wsl.append(Wga)
        Wsl.append(Wb[list(heads)])
        WprojT = np.ascontiguousarray(np.concatenate(Wsl, axis=0).T)
        consts = np.zeros((128, 32), np.float32)
        for i, h in enumerate(heads):
            consts[:, 12 * i + 0:12 * i + 4] = conv_q[128 * h:128 * (h + 1)]
            consts[:, 12 * i + 4:12 * i + 8] = conv_k[128 * h:128 * (h + 1)]
            consts[:, 12 * i + 8:12 * i + 12] = conv_v[128 * h:128 * (h + 1)]
            consts[:, 24 + i] = dt_bias[128 * h:128 * (h + 1)]
            consts[:, 26 + i] = -np.exp(A_log[h])
        consts[:, 28] = EPS
        consts[:, 29] = D * EPS
        wfb2 = np.concatenate([Wfb[128 * h:128 * (h + 1)].T for h in heads], axis=1)
        wgb2 = np.concatenate([Wgb[128 * h:128 * (h + 1)].T for h in heads], axis=1)
        in_maps.append({
            "xt": xT.astype(BF16),
            "wp": WprojT.astype(BF16),
            "wfb": np.ascontiguousarray(wfb2).astype(BF16),
            "wgb": np.ascontiguousarray(wgb2).astype(BF16),
            "wo": WoT_folded.astype(BF16),
            "cst": consts,
        })
    return in_maps


def _estimate_exec_ns(nc):
    """Best-effort single-core HW time estimate from the instruction cost
    model (NTFF profiling is unavailable under the axon client)."""
    try:
        from concourse.timeline_sim import TimelineSim
        return int(TimelineSim(nc, trace=False).simulate())
    except Exception:
        return None


def kernel(**inputs):
    global LAST_EXEC_NS
    from concourse.bass_utils import run_bass_kernel_spmd

    if "nc" not in _CACHE:
        _CACHE["nc"] = build_nc()
    nc = _CACHE["nc"]
    in_maps = _host_inputs(inputs)
    res = run_bass_kernel_spmd(nc, in_maps, core_ids=list(range(NCORES)), trace=False)
    if res.exec_time_ns is not None:
        LAST_EXEC_NS = res.exec_time_ns
    else:
        LAST_EXEC_NS = _estimate_exec_ns(nc)
    out = np.zeros((B * T, H), np.float32)
    for cidx in range(NCORES):
        r = res.results[cidx]["out"]
        out[128 * cidx:128 * (cidx + 1)] = r[0:128]
        out[T + 128 * cidx:T + 128 * (cidx + 1)] = r[128:256]
    return out

